# revision 1
# baseline (speedup 1.0000x reference)
"""Trainium2 Bass kernel for the EnhancedEncoderLayer (dense MHA + low-rank
top-k sparse attention + FFN, two layernorms).

Sharding: 8 cores = (batch b in 0..3) x (query-half h in {0,1}). Each core
computes output rows [b, h*512:(h+1)*512, :]. K/V-side projections are
computed redundantly per batch pair (no cross-core communication).

The host permutes src[b].T columns so each core's own query tokens are
columns 0..511 (attention contracts over all keys, so key order is
irrelevant); this keeps the SPMD program identical across cores.

Precision: trunk matmuls run as float32r (full fp32 streamed through 4
XBUSes, 1 cycle/row at N=512). The dense-attention operands (k, q, p, V)
are bf16 - their error is damped by the small dense-attention output scale.
The sparse top-k path stays f32r end-to-end (threshold sensitivity).

Schedule highlights:
- sparse scores + per-row top-k threshold bisection (DVE) are emitted first
  and overlap all dense-path PE work;
- k/q projections are interleaved per-head-pair with dense attention so the
  ACT-bound softmax exps overlap projection matmuls;
- spmm is qt-outer so the fuse+LN1 chain starts while spmm still runs.
"""
import sys
import os
import contextlib

for _p in ('/opt/trn_rl_repo',):
    if _p not in sys.path:
        sys.path.insert(0, _p)

import numpy as np
import concourse.bacc as bacc
import concourse.tile as tile
from concourse import mybir
from concourse.bass_utils import run_bass_kernel_spmd
from concourse.masks import make_identity

F32 = mybir.dt.float32
F32R = mybir.dt.float32r
BF16 = mybir.dt.bfloat16
AF = mybir.ActivationFunctionType
OP = mybir.AluOpType

B, S, D, H, R, DFF = 4, 1024, 1024, 16, 64, 4096
DH = D // H          # 64
SQ = S // 2          # 512 own queries per core
KK = max(1, int(S * 0.2))   # 204
KC = D // 128        # 8 contraction chunks over D
FC = DFF // 128      # 32 chunks over DFF
NQT = SQ // 128      # 4 query tiles
NTOK = S // 128      # 8 token tiles
BISECT_ITERS = 28
INV_SQRT = 0.125     # 1/sqrt(DH) == 1/sqrt(R)

_cached = {}


def _build():
    nc = bacc.Bacc()

    def din(name, shape):
        return nc.declare_dram_parameter(name, list(shape), F32, isOutput=False)

    xT = din("xT", [D, S])          # src[b].T, own-query columns first
    x_own = din("x_own", [SQ, D])   # own rows, token-major
    wqkvT = din("wqkvT", [D, 3 * D])
    woT = din("woT", [D, D])
    vpT = din("vpT", [D, D])
    qkpT = din("qkpT", [D, 2 * R])
    f1T = nc.declare_dram_parameter("f1T", [D, DFF], BF16, isOutput=False)
    f2T = nc.declare_dram_parameter("f2T", [DFF, D], BF16, isOutput=False)
    bqkv = din("bqkv", [3 * D])
    bo = din("bo", [D])
    bvp = din("bvp", [D])
    bqp = din("bqp", [R])
    bkp = din("bkp", [R])
    b1 = din("b1", [DFF])
    b2 = din("b2", [D])
    g1 = din("g1", [D])
    be1 = din("be1", [D])
    g2 = din("g2", [D])
    be2 = din("be2", [D])
    lam = din("lam", [1, 1])
    out = nc.declare_dram_parameter("out", [SQ, D], F32, isOutput=True)
    DBG = bool(os.environ.get("BASSK_DEBUG"))
    if DBG:
        dbg_dense = nc.declare_dram_parameter("dbg_dense", [SQ, D], F32,
                                              isOutput=True)
        dbg_sparse = nc.declare_dram_parameter("dbg_sparse", [SQ, D], F32,
                                               isOutput=True)
        dbg_lo = nc.declare_dram_parameter("dbg_lo", [128, NQT], F32,
                                           isOutput=True)
        dbg_rs = nc.declare_dram_parameter("dbg_rs", [128, NQT], F32,
                                           isOutput=True)

    xT_r = xT.ap().bitcast(F32R).rearrange("(kc p) s -> p kc s", p=128)
    wqkvT_r = wqkvT.ap().bitcast(F32R).rearrange("(kc p) f -> p kc f", p=128)
    woT_r = woT.ap().bitcast(F32R).rearrange("(kc p) f -> p kc f", p=128)
    vpT_r = vpT.ap().bitcast(F32R).rearrange("(kc p) f -> p kc f", p=128)
    qkpT_r = qkpT.ap().bitcast(F32R).rearrange("(kc p) f -> p kc f", p=128)
    f1T_r = f1T.ap().rearrange("(kc p) f -> p kc f", p=128)
    f2T_r = f2T.ap().rearrange("(kc p) f -> p kc f", p=128)

    with tile.TileContext(nc) as tc:
        est = contextlib.ExitStack()
        with est:
            # ---------------- constants ----------------
            consts = est.enter_context(tc.tile_pool(name="consts", bufs=1))

            ident_f = consts.tile([128, 128], F32, name="ident_f")
            make_identity(nc, ident_f)
            ident_r = consts.tile([128, 128], F32R, name="ident_r")
            nc.vector.tensor_copy(out=ident_r, in_=ident_f)

            eps_t = consts.tile([128, 1], F32, name="eps_t")
            nc.vector.memset(eps_t, 1e-5)
            ones1 = consts.tile([128, 1], F32, name="ones1")
            nc.vector.memset(ones1, 1.0)
            ones16 = consts.tile([128, 16], F32, name="ones16")
            nc.vector.memset(ones16, 1.0)

            lam_t = consts.tile([1, 1], F32, name="lam_t")
            nc.sync.dma_start(out=lam_t, in_=lam.ap())
            sg_t = consts.tile([1, 1], F32, name="sg_t")
            nc.scalar.activation(out=sg_t, in_=lam_t, func=AF.Sigmoid)
            sig_bc = consts.tile([128, 1], F32, name="sig_bc")
            nc.gpsimd.partition_broadcast(sig_bc, sg_t)
            oms_bc = consts.tile([128, 1], F32, name="oms_bc")
            nc.vector.tensor_sub(oms_bc, ones1, sig_bc)

            bqkv_c = consts.tile([128, 24], F32, name="bqkv_c")
            bvp_c = consts.tile([128, 8], F32, name="bvp_c")
            bqp_c = consts.tile([64, 1], F32, name="bqp_c")
            bkp_c = consts.tile([64, 1], F32, name="bkp_c")
            b1_c = consts.tile([128, 32], F32, name="b1_c")
            g1_c = consts.tile([128, 8], F32, name="g1_c")
            be1_c = consts.tile([128, 8], F32, name="be1_c")

            def load_bias_cols():
                # strided gathers with expensive descriptor generation: on
                # the Activation queue, after the xT chunks
                nc.scalar.dma_start(
                    out=bqkv_c,
                    in_=bqkv.ap().rearrange("(c p) -> p c", p=128))
                nc.scalar.dma_start(
                    out=bvp_c, in_=bvp.ap().rearrange("(c p) -> p c", p=128))
                nc.scalar.dma_start(
                    out=bqp_c, in_=bqp.ap().rearrange("(c p) -> p c", p=64))
                nc.scalar.dma_start(
                    out=bkp_c, in_=bkp.ap().rearrange("(c p) -> p c", p=64))
                nc.scalar.dma_start(
                    out=b1_c, in_=b1.ap().rearrange("(c p) -> p c", p=128))
                nc.scalar.dma_start(
                    out=g1_c, in_=g1.ap().rearrange("(c p) -> p c", p=128))
                nc.scalar.dma_start(
                    out=be1_c, in_=be1.ap().rearrange("(c p) -> p c", p=128))

            # broadcast bias (sig * out_proj_b) built at t=0
            bo_sig = consts.tile([128, D], F32, name="bo_sig")
            bo_row = consts.tile([1, D], F32, name="bo_row")
            nc.sync.dma_start(out=bo_row,
                              in_=bo.ap().rearrange("(o d) -> o d", o=1))
            nc.gpsimd.partition_broadcast(bo_sig, bo_row)
            nc.vector.tensor_scalar_mul(bo_sig, bo_sig, sig_bc)

            # own-token residual (+ sig*bo); loaded after xT is in flight
            xot_pool = est.enter_context(tc.tile_pool(name="xot_pool",
                                                      bufs=1))
            xot = xot_pool.tile([128, NQT, D], F32, name="xot")

            bis = est.enter_context(tc.tile_pool(name="bis", bufs=1))
            lo = bis.tile([128, NQT], F32, name="lo")
            hi = bis.tile([128, NQT], F32, name="hi")
            mid = bis.tile([128, NQT], F32, name="mid")
            cnts = bis.tile([128, NQT], F32, name="cnts")
            pred = bis.tile([128, NQT], mybir.dt.uint32, name="pred")
            rs_sp = bis.tile([128, NQT], F32, name="rs_sp")
            rcp_sp = bis.tile([128, NQT], F32, name="rcp_sp")

            # long-lived activation groups (left stack)
            sp_stack = contextlib.ExitStack()
            sp_pool = sp_stack.enter_context(
                tc.tile_pool(name="sp_pool", bufs=1))
            Vsp = sp_pool.tile([128, NTOK, D], F32R, name="Vsp")
            kspT = sp_pool.tile([64, S], F32R, name="kspT")
            qspT = sp_pool.tile([64, SQ], F32R, name="qspT")

            dn_stack = contextlib.ExitStack()
            dn_pool = dn_stack.enter_context(
                tc.tile_pool(name="dn_pool", bufs=1))
            kT = dn_pool.tile([128, KC, S], BF16, name="kT")
            Vaug = dn_pool.tile([128, NTOK, H * (DH + 1)], BF16, name="Vaug")
            qT = dn_pool.tile([128, KC, SQ], BF16, name="qT")

            Vaug_h = Vaug.rearrange("p t (h c) -> p t h c", c=DH + 1)
            for t in range(NTOK):
                nc.vector.tensor_copy(out=Vaug_h[:, t, :, DH:DH + 1],
                                      in_=ones16)

            # right-stack pools (all close together after phase 7)
            psp_stack = contextlib.ExitStack()
            psp_pool = psp_stack.enter_context(
                tc.tile_pool(name="psp_pool", bufs=1, side="right"))
            psp = [psp_pool.tile([128, S], F32, name=f"psp{qt}")
                   for qt in range(NQT)]
            scr_stack = contextlib.ExitStack()
            scr_pool = scr_stack.enter_context(
                tc.tile_pool(name="scr", bufs=1, side="right"))
            ctx_stack = contextlib.ExitStack()
            ctx_pool = ctx_stack.enter_context(
                tc.tile_pool(name="ctx_pool", bufs=1, side="right"))
            ctxT = ctx_pool.tile([128, KC, SQ], F32R, name="ctxT")
            # ============ projections + sparse path + attention ============
            with contextlib.ExitStack() as ph0:
                xt_pool = ph0.enter_context(
                    tc.tile_pool(name="xt_pool", bufs=1))
                wstr = ph0.enter_context(tc.tile_pool(name="wstr", bufs=2))
                vtmp = ph0.enter_context(tc.tile_pool(name="vtmp", bufs=1))
                pt_pool = ph0.enter_context(
                    tc.tile_pool(name="pt_pool", bufs=8))
                rc_pool = ph0.enter_context(
                    tc.tile_pool(name="rc_pool", bufs=1))
                ps_proj = ph0.enter_context(
                    tc.tile_pool(name="ps_proj", bufs=2, space="PSUM"))
                ps_attn = ph0.enter_context(
                    tc.tile_pool(name="ps_attn", bufs=2, space="PSUM"))
                ps_ctx = ph0.enter_context(
                    tc.tile_pool(name="ps_ctx", bufs=2, space="PSUM"))

                # small sparse weights first, then xT on both queues
                qkpt = wstr.tile([128, KC, 2 * R], F32R, name="qkpt",
                                 tag="wsmall")
                nc.sync.dma_start(out=qkpt, in_=qkpT_r)
                qpt = qkpt[:, :, 0:R]
                kpt = qkpt[:, :, R:2 * R]
                xTt = xt_pool.tile([128, KC, S], F32R, name="xTt")
                for kc in range(KC):
                    eng = nc.scalar if kc % 2 == 0 else nc.sync
                    eng.dma_start(out=xTt[:, kc, :], in_=xT_r[:, kc, :])
                load_bias_cols()

                # ---- sparse projections + scores ----
                with nc.named_scope("p0_ksp_qsp"):
                    for nh in range(2):
                        ps = ps_proj.tile([128, 512], F32, name="ps", tag="ps")
                        for kc in range(KC):
                            nc.tensor.matmul(
                                ps[0:64, :], kpt[:, kc, :],
                                xTt[:, kc, nh * 512:nh * 512 + 512],
                                start=(kc == 0), stop=(kc == KC - 1))
                        nc.scalar.activation(
                            out=kspT[:, nh * 512:nh * 512 + 512],
                            in_=ps[0:64, :], func=AF.Identity, bias=bkp_c,
                            scale=1.0)
                    ps = ps_proj.tile([128, 512], F32, name="ps", tag="ps")
                    for kc in range(KC):
                        nc.tensor.matmul(ps[0:64, :], qpt[:, kc, :],
                                         xTt[:, kc, 0:SQ],
                                         start=(kc == 0), stop=(kc == KC - 1))
                    nc.scalar.activation(out=qspT, in_=ps[0:64, :],
                                         func=AF.Identity, bias=bqp_c,
                                         scale=1.0)

                with nc.named_scope("p2_ssp"):
                    for qt in range(NQT):
                        for nh in range(2):
                            ps = ps_proj.tile([128, 512], F32, name="ps",
                                              tag="ps")
                            nc.tensor.matmul(
                                ps, qspT[:, qt * 128:qt * 128 + 128],
                                kspT[:, nh * 512:nh * 512 + 512],
                                start=True, stop=True)
                            nc.scalar.activation(
                                out=psp[qt][:, nh * 512:nh * 512 + 512],
                                in_=ps, func=AF.Exp, scale=INV_SQRT)

                # ---- top-k threshold bisection (DVE; overlaps PE below) ----
                with nc.named_scope("p3_bisect"):
                    nc.vector.memset(lo, 0.0)
                    nc.vector.memset(hi, 16.0)
                    for it in range(BISECT_ITERS):
                        nc.vector.tensor_add(mid, lo, hi)
                        nc.vector.tensor_scalar_mul(mid, mid, 0.5)
                        for qt in range(NQT):
                            scr = scr_pool.tile([128, S], F32, name="scr",
                                                tag="scr")
                            nc.vector.scalar_tensor_tensor(
                                out=scr, in0=psp[qt],
                                scalar=mid[:, qt:qt + 1],
                                in1=ones1.to_broadcast([128, S]),
                                op0=OP.is_ge, op1=OP.mult,
                                accum_out=cnts[:, qt:qt + 1])
                        nc.vector.tensor_scalar(out=pred, in0=cnts,
                                                scalar1=float(KK),
                                                scalar2=None, op0=OP.is_ge)
                        nc.vector.copy_predicated(lo, pred, mid)
                        nc.vector.tensor_scalar(out=pred, in0=cnts,
                                                scalar1=float(KK),
                                                scalar2=None, op0=OP.is_lt)
                        nc.vector.copy_predicated(hi, pred, mid)
                    for qt in range(NQT):
                        nc.vector.scalar_tensor_tensor(
                            out=psp[qt], in0=psp[qt],
                            scalar=lo[:, qt:qt + 1],
                            in1=psp[qt], op0=OP.is_ge, op1=OP.mult,
                            accum_out=rs_sp[:, qt:qt + 1])
                    if DBG:
                        nc.sync.dma_start(out=dbg_lo.ap(), in_=lo)
                        nc.sync.dma_start(out=dbg_rs.ap(), in_=rs_sp)
                    nc.vector.tensor_scalar(out=rs_sp, in0=rs_sp,
                                            scalar1=1e-9, scalar2=None,
                                            op0=OP.add)
                    nc.vector.reciprocal(rcp_sp, rs_sp)
                    nc.vector.tensor_scalar_mul(rcp_sp, rcp_sp, oms_bc)

                for qt in range(NQT):
                    nc.scalar.dma_start(
                        out=xot[:, qt, :],
                        in_=x_own.ap()[qt * 128:qt * 128 + 128, :])
                    nc.gpsimd.tensor_add(xot[:, qt, :], xot[:, qt, :],
                                         bo_sig)

                def w_chunk(w_view, f0, nfs=128):
                    wt = wstr.tile([128, KC, 128], F32R, name="wt", tag="wt")
                    nc.sync.dma_start(out=wt[:, :, :nfs],
                                      in_=w_view[:, :, f0:f0 + nfs])
                    return wt

                # ---- v / vsp projections -> token-major Vaug / Vsp ----
                def proj_transpose(w_view, f_lo, bias_col, bias_lo, to_vaug,
                                   scope):
                    with nc.named_scope(scope):
                        for ft in range(8):
                            wt = w_chunk(w_view, f_lo + ft * 128)
                            if True:
                                vt = vtmp.tile([128, S], F32R, name="vt",
                                               tag="vt")
                                for nh in range(2):
                                    ps = ps_proj.tile([128, 512], F32,
                                                      name="ps", tag="ps")
                                    for kc in range(KC):
                                        nc.tensor.matmul(
                                            ps,
                                            wt[:, kc, 0:128],
                                            xTt[:, kc,
                                                nh * 512:nh * 512 + 512],
                                            start=(kc == 0),
                                            stop=(kc == KC - 1))
                                    nc.scalar.activation(
                                        out=vt[:, nh * 512:nh * 512 + 512],
                                        in_=ps, func=AF.Identity,
                                        bias=bias_col[:, bias_lo + ft:
                                                      bias_lo + ft + 1],
                                        scale=1.0)
                                for t in range(NTOK):
                                    pst = ps_ctx.tile([128, 128], F32R,
                                                      name="pst", tag="pst",
                                                      bufs=2)
                                    nc.tensor.transpose(
                                        pst, vt[:, t * 128:t * 128 + 128],
                                        ident_r)
                                    if to_vaug:
                                        h0 = 2 * ft
                                        nc.scalar.copy(
                                            out=Vaug_h[:, t, h0, 0:DH],
                                            in_=pst[:, 0:64])
                                        nc.scalar.copy(
                                            out=Vaug_h[:, t, h0 + 1, 0:DH],
                                            in_=pst[:, 64:128])
                                    else:
                                        nc.scalar.copy(
                                            out=Vsp[:, t,
                                                    ft * 128:ft * 128 + 128],
                                            in_=pst)

                proj_transpose(wqkvT_r, 2 * D, bqkv_c, 16, True, "p0_v")
                proj_transpose(vpT_r, 0, bvp_c, 0, False, "p0_vsp")

                # ---- interleaved k/q projections + dense attention ----
                with nc.named_scope("p4_kq_attn"):
                    for jj in range(4):
                        for fi in range(2):
                            ft = jj * 2 + fi
                            wkc = w_chunk(wqkvT_r, D + ft * 128)
                            for nh in range(2):
                                ps = ps_proj.tile([128, 512], F32, name="ps",
                                                  tag="ps")
                                for kc in range(KC):
                                    nc.tensor.matmul(
                                        ps,
                                        wkc[:, kc, 0:128],
                                        xTt[:, kc, nh * 512:nh * 512 + 512],
                                        start=(kc == 0), stop=(kc == KC - 1))
                                nc.scalar.activation(
                                    out=kT[:, ft, nh * 512:nh * 512 + 512],
                                    in_=ps, func=AF.Identity,
                                    bias=bqkv_c[:, 8 + ft:8 + ft + 1],
                                    scale=1.0)
                            wqc = w_chunk(wqkvT_r, ft * 128)
                            ps = ps_proj.tile([128, 512], F32, name="ps",
                                              tag="ps")
                            for kc in range(KC):
                                nc.tensor.matmul(
                                    ps, wqc[:, kc, 0:128],
                                    xTt[:, kc, 0:SQ],
                                    start=(kc == 0), stop=(kc == KC - 1))
                            nc.scalar.activation(
                                out=qT[:, ft, :], in_=ps, func=AF.Identity,
                                bias=bqkv_c[:, ft:ft + 1], scale=1.0)
                        # attention for the 4 heads of these two f-tiles
                        for hh in range(4 * jj, 4 * jj + 4):
                            po = 64 * (hh % 2)
                            ft = hh // 2
                            pTs = []
                            for t in range(NTOK):
                                ps = ps_attn.tile([128, 512], F32,
                                                  name="ps_s", tag="ps_s")
                                nc.tensor.matmul(
                                    ps,
                                    kT[po:po + 64, ft, t * 128:t * 128 + 128],
                                    qT[po:po + 64, ft, :],
                                    start=True, stop=True)
                                pt = pt_pool.tile([128, 512], BF16, name="pT",
                                                  tag="pT")
                                nc.scalar.activation(out=pt, in_=ps,
                                                     func=AF.Exp,
                                                     scale=INV_SQRT)
                                pTs.append(pt)
                            pctx = ps_ctx.tile([128, 512], F32, name="ps_c",
                                               tag="ps_c")
                            for t in range(NTOK):
                                nc.tensor.matmul(
                                    pctx[0:65, :],
                                    Vaug[:, t, hh * 65:hh * 65 + 65],
                                    pTs[t], start=(t == 0),
                                    stop=(t == NTOK - 1))
                            rsr = rc_pool.tile([1, 512], F32, name="rsr",
                                               tag="rsr")
                            nc.vector.tensor_copy(out=rsr,
                                                  in_=pctx[64:65, :])
                            rch = rc_pool.tile([1, 512], F32, name="rch",
                                               tag="rch")
                            nc.vector.reciprocal_approx_fast(out=rch,
                                                             in_=rsr)
                            rb = rc_pool.tile([64, 512], F32, name="rb",
                                              tag="rb")
                            nc.gpsimd.partition_broadcast(rb, rch)
                            nc.vector.tensor_mul(
                                out=ctxT[po:po + 64, ft, :],
                                in0=pctx[0:64, :], in1=rb)

            dn_stack.close()   # free kT, Vaug, qT

            ds_stack = contextlib.ExitStack()
            ds_pool = ds_stack.enter_context(
                tc.tile_pool(name="ds_pool", bufs=1, side="right"))
            dense_s = ds_pool.tile([128, NQT, D], F32, name="dense_s")
            sparse_s = ds_pool.tile([128, NQT, D], F32, name="sparse_s")

            # ============ out_proj ============
            with contextlib.ExitStack() as ph5:
                w2str = ph5.enter_context(tc.tile_pool(name="w2str", bufs=3))
                pm_pool = ph5.enter_context(
                    tc.tile_pool(name="pm_pool", bufs=1))
                ps_mm = ph5.enter_context(
                    tc.tile_pool(name="ps_mm", bufs=4, space="PSUM"))
                ps_tr2 = ph5.enter_context(
                    tc.tile_pool(name="ps_tr2", bufs=2, space="PSUM"))
                with nc.named_scope("p5_outproj"):
                    for nh in range(2):
                        pss = [ps_mm.tile([128, 512], F32, name="ps_o",
                                          tag="ps_o") for _ in range(NQT)]
                        for kc in range(KC):
                            wo_t = w2str.tile([128, 512], F32R, name="wo_t",
                                              tag="w2")
                            nc.sync.dma_start(
                                out=wo_t,
                                in_=woT_r[:, kc, nh * 512:nh * 512 + 512])
                            for qt in range(NQT):
                                nc.tensor.matmul(
                                    pss[qt],
                                    ctxT[:, kc, qt * 128:qt * 128 + 128],
                                    wo_t, start=(kc == 0),
                                    stop=(kc == KC - 1))
                        for qt in range(NQT):
                            nc.scalar.activation(
                                out=dense_s[:, qt, nh * 512:nh * 512 + 512],
                                in_=pss[qt], func=AF.Copy, scale=sig_bc)

                # ===== masked-p transpose + sparse attn @ Vsp (qt-outer) ====
                pmT = pm_pool.tile([128, NTOK, SQ], F32R, name="pmT")
                with nc.named_scope("p6_spmm"):
                    for qt in range(NQT):
                        for t in range(NTOK):
                            pst = ps_tr2.tile([128, 128], F32, name="pst2",
                                              tag="pst2")
                            nc.tensor.transpose(
                                pst, psp[qt][:, t * 128:t * 128 + 128],
                                ident_f)
                            nc.scalar.copy(
                                out=pmT[:, t, qt * 128:qt * 128 + 128],
                                in_=pst)
                        for nh in range(2):
                            ps = ps_mm.tile([128, 512], F32, name="ps_o",
                                            tag="ps_o")
                            for t in range(NTOK):
                                nc.tensor.matmul(
                                    ps,
                                    pmT[:, t, qt * 128:qt * 128 + 128],
                                    Vsp[:, t, nh * 512:nh * 512 + 512],
                                    start=(t == 0), stop=(t == NTOK - 1))
                            nc.scalar.activation(
                                out=sparse_s[:, qt, nh * 512:nh * 512 + 512],
                                in_=ps, func=AF.Copy,
                                scale=rcp_sp[:, qt:qt + 1])
            sp_stack.close()

            # ============ fuse + LN1 ============
            fse = est.enter_context(tc.tile_pool(name="fse", bufs=1))
            x78_stack = contextlib.ExitStack()
            x78 = x78_stack.enter_context(tc.tile_pool(name="x78", bufs=1))

            def bcast_row(src_ap, name, rp):
                t = fse.tile([128, D], F32, name=name)
                row = rp.tile([1, D], F32, name="row", tag="row")
                nc.sync.dma_start(out=row,
                                  in_=src_ap.rearrange("(o d) -> o d", o=1))
                nc.gpsimd.partition_broadcast(t, row)
                return t

            with contextlib.ExitStack() as rows:
                rp = rows.enter_context(tc.tile_pool(name="rp", bufs=2))
                b12_bc = bcast_row(b2.ap(), "b12_bc", rp)
                be1_t = bcast_row(be1.ap(), "be1_t", rp)
                nc.vector.tensor_add(b12_bc, b12_bc, be1_t)
                g1_bc = bcast_row(g1.ap(), "g1_bc", rp)
                g2_bc = bcast_row(g2.ap(), "g2_bc", rp)
                be2_bc = bcast_row(be2.ap(), "be2_bc", rp)

            xhat = x78.tile([128, NQT, D], F32, name="xhat")
            xg = fse.tile([128, NQT, D], F32, name="xg")
            stats = fse.tile([128, NQT, 2, 6], F32, name="stats")
            mv2 = fse.tile([128, NQT, 2], F32, name="mv2")
            sd = fse.tile([128, NQT], F32, name="sd")
            rstd = fse.tile([128, NQT], F32, name="rstd")

            def ln_normalize(x1, qt):
                for half in range(2):
                    nc.vector.bn_stats(
                        out=stats[:, qt, half, :],
                        in_=x1[:, half * 512:half * 512 + 512])
                nc.vector.bn_aggr(out=mv2[:, qt, :], in_=stats[:, qt])
                nc.scalar.activation(out=sd[:, qt:qt + 1],
                                     in_=mv2[:, qt, 1:2], func=AF.Sqrt,
                                     bias=eps_t, scale=1.0)
                nc.vector.reciprocal(rstd[:, qt:qt + 1], sd[:, qt:qt + 1])
                nc.vector.tensor_scalar(out=x1, in0=x1,
                                        scalar1=mv2[:, qt, 0:1],
                                        scalar2=rstd[:, qt:qt + 1],
                                        op0=OP.subtract, op1=OP.mult)

            with nc.named_scope("p7_fuse_ln1"):
                for qt in range(NQT):
                    x1 = xhat[:, qt, :]
                    nc.vector.tensor_add(x1, dense_s[:, qt, :],
                                         sparse_s[:, qt, :])
                    nc.vector.tensor_add(x1, x1, xot[:, qt, :])
                    ln_normalize(x1, qt)
                for qt in range(NQT):
                    nc.vector.tensor_mul(xg[:, qt, :], xhat[:, qt, :], g1_bc)
                    nc.vector.tensor_add(xg[:, qt, :], xg[:, qt, :], b12_bc)
            if DBG:
                for qt in range(NQT):
                    nc.sync.dma_start(
                        out=dbg_dense.ap()[qt * 128:qt * 128 + 128, :],
                        in_=dense_s[:, qt, :])
                    nc.sync.dma_start(
                        out=dbg_sparse.ap()[qt * 128:qt * 128 + 128, :],
                        in_=sparse_s[:, qt, :])
            ds_stack.close()
            ctx_stack.close()
            scr_stack.close()
            psp_stack.close()

            # ============ xhat transpose -> ff1 input ============
            xln_stack = contextlib.ExitStack()
            xln_pool = xln_stack.enter_context(
                tc.tile_pool(name="xln_pool", bufs=1, side="right"))
            xlnT = xln_pool.tile([128, KC, SQ], BF16, name="xlnT")
            with contextlib.ExitStack() as ph8:
                ps_tr3 = ph8.enter_context(
                    tc.tile_pool(name="ps_tr3", bufs=2, space="PSUM"))
                with nc.named_scope("p8_xT"):
                    for qt in range(NQT):
                        for fc in range(KC):
                            pst = ps_tr3.tile([128, 128], F32, name="pst3",
                                              tag="pst3")
                            nc.tensor.transpose(
                                pst, xhat[:, qt, fc * 128:fc * 128 + 128],
                                ident_f)
                            nc.vector.tensor_scalar(
                                out=xlnT[:, fc, qt * 128:qt * 128 + 128],
                                in0=pst,
                                scalar1=g1_c[:, fc:fc + 1],
                                scalar2=be1_c[:, fc:fc + 1],
                                op0=OP.mult, op1=OP.add)
            x78_stack.close()

            # ============ ff1 + relu ============
            h1_stack = contextlib.ExitStack()
            h1_pool = h1_stack.enter_context(
                tc.tile_pool(name="h1_pool", bufs=1))
            h1T = h1_pool.tile([128, FC, SQ], BF16, name="h1T")
            with contextlib.ExitStack() as ph9:
                w3str = ph9.enter_context(tc.tile_pool(name="w3str", bufs=2))
                ps_f1 = ph9.enter_context(
                    tc.tile_pool(name="ps_f1", bufs=4, space="PSUM"))
                with nc.named_scope("p9_ff1"):
                    for jj in range(16):
                        wt = w3str.tile([128, KC, 256], BF16, name="w1t",
                                        tag="w3")
                        f0 = jj * 256
                        eng = nc.scalar if jj % 2 == 0 else nc.sync
                        eng.dma_start(out=wt, in_=f1T_r[:, :, f0:f0 + 256])
                        for fi in range(2):
                            dft = jj * 2 + fi
                            ps = ps_f1.tile([128, 512], F32, name="ps_f",
                                            tag="ps_f")
                            for kc in range(KC):
                                nc.tensor.matmul(
                                    ps, wt[:, kc, fi * 128:fi * 128 + 128],
                                    xlnT[:, kc, :],
                                    start=(kc == 0), stop=(kc == KC - 1))
                            nc.scalar.activation(
                                out=h1T[:, dft, :], in_=ps, func=AF.Relu,
                                bias=b1_c[:, dft:dft + 1], scale=1.0)
            xln_stack.close()

            # ============ ff2 + residual + LN2 + out ============
            ff_s = fse.tile([128, NQT, D], F32, name="ff_s")
            with contextlib.ExitStack() as ph10:
                w4str = ph10.enter_context(tc.tile_pool(name="w4str", bufs=4))
                ps_f2 = ph10.enter_context(
                    tc.tile_pool(name="ps_f2", bufs=4, space="PSUM"))
                with nc.named_scope("p10_ff2"):
                    for nh in range(2):
                        pss = [ps_f2.tile([128, 512], F32, name="ps_g",
                                          tag="ps_g") for _ in range(NQT)]
                        for kc in range(FC):
                            f2h = w4str.tile([128, 512], BF16, name="f2h",
                                             tag="w4")
                            eng = nc.scalar if kc % 2 == 0 else nc.sync
                            eng.dma_start(
                                out=f2h,
                                in_=f2T_r[:, kc, nh * 512:nh * 512 + 512])
                            for qt in range(NQT):
                                nc.tensor.matmul(
                                    pss[qt],
                                    h1T[:, kc, qt * 128:qt * 128 + 128],
                                    f2h, start=(kc == 0), stop=(kc == FC - 1))
                        for qt in range(NQT):
                            nc.scalar.copy(
                                out=ff_s[:, qt, nh * 512:nh * 512 + 512],
                                in_=pss[qt])

                with nc.named_scope("p10_ln2"):
                    for qt in range(NQT):
                        x2 = ff_s[:, qt, :]
                        nc.vector.tensor_add(x2, x2, xg[:, qt, :])
                        ln_normalize(x2, qt)
                        ot = fse.tile([128, D], F32, name="out_t",
                                      tag="out_t", bufs=2)
                        nc.vector.tensor_mul(ot, x2, g2_bc)
                        nc.vector.tensor_add(ot, ot, be2_bc)
                        nc.scalar.dma_start(
                            out=out.ap()[qt * 128:qt * 128 + 128, :], in_=ot)
            h1_stack.close()

    nc.compile()
    return nc


def _prep_inputs(src, in_proj_w, in_proj_b, out_proj_w, out_proj_b,
                 Qp_w, Qp_b, Kp_w, Kp_b, Vp_w, Vp_b, lam,
                 ff1_w, ff1_b, ff2_w, ff2_b, ln1_g, ln1_b, ln2_g, ln2_b):
    import ml_dtypes
    f = np.float32
    A = lambda x: np.ascontiguousarray(x, dtype=f)
    AB = lambda x: np.ascontiguousarray(np.asarray(x, dtype=f),
                                        dtype=ml_dtypes.bfloat16)
    shared = {
        "wqkvT": A(np.asarray(in_proj_w).T),
        "woT": A(np.asarray(out_proj_w).T),
        "vpT": A(np.asarray(Vp_w).T),
        "qkpT": A(np.concatenate([np.asarray(Qp_w).T, np.asarray(Kp_w).T],
                                 axis=1)),
        "f1T": AB(np.asarray(ff1_w).T),
        "f2T": AB(np.asarray(ff2_w).T), "bqkv": A(in_proj_b),
        "bo": A(out_proj_b), "bvp": A(Vp_b), "bqp": A(Qp_b), "bkp": A(Kp_b),
        "b1": A(ff1_b), "b2": A(ff2_b), "g1": A(ln1_g), "be1": A(ln1_b),
        "g2": A(ln2_g), "be2": A(ln2_b),
        "lam": A(np.asarray(lam)).reshape(1, 1),
    }
    in_maps = []
    for core in range(8):
        b, h = core // 2, core % 2
        srcb = np.asarray(src[b])
        xTb = srcb.T
        if h == 1:
            # own-query columns first (key order is irrelevant to attention)
            xTb = np.concatenate([xTb[:, SQ:], xTb[:, :SQ]], axis=1)
        m = dict(shared)
        m["xT"] = A(xTb)
        m["x_own"] = A(srcb[h * SQ:(h + 1) * SQ, :])
        in_maps.append(m)
    return in_maps


def _run(inputs, trace=False):
    if "nc" not in _cached:
        _cached["nc"] = _build()
    nc = _cached["nc"]
    in_maps = _prep_inputs(**inputs)
    res = run_bass_kernel_spmd(nc, in_maps, core_ids=list(range(8)),
                               trace=trace)
    out = np.empty((B, S, D), np.float32)
    for core in range(8):
        b, h = core // 2, core % 2
        out[b, h * SQ:(h + 1) * SQ, :] = res.results[core]["out"]
    return out, res


def kernel(**inputs) -> np.ndarray:
    out, _ = _run(inputs, trace=False)
    return out



# revision 17
# speedup vs baseline: 1.0089x; 1.0089x over previous
"""Trainium2 Bass kernel for the EnhancedEncoderLayer (dense MHA + low-rank
top-k sparse attention + FFN, two layernorms).

Sharding: 8 cores = (batch b in 0..3) x (query-half h in {0,1}). Each core
computes output rows [b, h*512:(h+1)*512, :]. K/V-side projections are
computed redundantly per batch pair (no cross-core communication).

The host permutes src[b].T columns so each core's own query tokens are
columns 0..511 (attention contracts over all keys, so key order is
irrelevant); this keeps the SPMD program identical across cores.

v2 design notes:
- v/vsp projections are x-stationary (lhsT = xT chunk), producing
  token-major Vaug/Vsp directly -- no PE transposes, no ACT copies.
- dense attention is software-pipelined: scores(h) / proj filler /
  ctx(h-1), so ACT exp latency never stalls the in-order PE queue.
- the sparse path runs in bf16: exp writes bf16 psp, the top-k threshold
  bisection scans at 2x DVE rate with 18 iterations, spmm is bf16.
- outproj+spmm+fuse+LN1 run qt-outer so LN1 overlaps matmuls; ff2 is
  qt-outer against an SBUF-resident f2T with LN2+output pipelined per qt.
- ln1 gamma/beta are folded into ff1 weights/bias host-side.
- all host tensors are laid out so every DMA is contiguous per partition.
"""
import sys
import os
import contextlib

for _p in ('/opt/trn_rl_repo',):
    if _p not in sys.path:
        sys.path.insert(0, _p)

import numpy as np
import concourse.bacc as bacc
import concourse.tile as tile
from concourse import mybir
from concourse.bass_utils import run_bass_kernel_spmd
from concourse.masks import make_identity

F32 = mybir.dt.float32
F32R = mybir.dt.float32r
BF16 = mybir.dt.bfloat16
AF = mybir.ActivationFunctionType
OP = mybir.AluOpType

B, S, D, H, R, DFF = 4, 1024, 1024, 16, 64, 4096
DH = D // H          # 64
SQ = S // 2          # 512 own queries per core
KK = max(1, int(S * 0.2))   # 204
KC = D // 128        # 8 contraction chunks over D
FC = DFF // 128      # 32 chunks over DFF
NQT = SQ // 128      # 4 query tiles
NTOK = S // 128      # 8 token tiles
BISECT_ITERS = 19
INV_SQRT = 0.125     # 1/sqrt(DH) == 1/sqrt(R)

_cached = {}


def _build():
    nc = bacc.Bacc()

    def din(name, shape, dt=F32):
        return nc.declare_dram_parameter(name, list(shape), dt, isOutput=False)

    xT = din("xT", [D, S])            # src[b].T, own-query columns first
    x_own = din("x_own", [SQ, D])     # own rows, token-major
    wqkvT = din("wqkvT", [D, 3 * D], BF16)
    woT = din("woT", [D, D], BF16)
    vpT = din("vpT", [D, D], BF16)
    qkp_pm = din("qkp_pm", [128, KC * 2 * R])   # partition-major packed
    f1T = din("f1T", [D, DFF], BF16)  # pre-scaled by ln1 gamma
    f2T = din("f2T", [DFF, D], BF16)
    # packed per-partition bias columns: [0:8]=q, [8:16]=k, [16:48]=b1_eff,
    # [48]=bqp (rows 0:64), [49]=bkp (rows 0:64)
    bcols = din("bcols", [128, 50])
    # packed broadcast rows: [bv, bvp, bo, b12, g1, g2, be2]
    brow = din("brow", [1, 7 * D])
    lam = din("lam", [1, 1])
    out = nc.declare_dram_parameter("out", [SQ, D], F32, isOutput=True)

    xT_r = xT.ap().bitcast(F32R).rearrange("(kc p) s -> p kc s", p=128)
    wqkvT_r = wqkvT.ap().rearrange("(kc p) f -> p kc f", p=128)
    woT_r = woT.ap().rearrange("(kc p) f -> p kc f", p=128)
    vpT_r = vpT.ap().rearrange("(kc p) f -> p kc f", p=128)
    qkp_r = qkp_pm.ap().bitcast(F32R).rearrange("p (kc f) -> p kc f", f=2 * R)
    f1T_r = f1T.ap().rearrange("(kc p) f -> p kc f", p=128)
    f2T_r = f2T.ap().rearrange("(kc p) f -> p kc f", p=128)

    with tile.TileContext(nc) as tc:
        est = contextlib.ExitStack()
        with est:
            # ---------------- constants ----------------
            consts = est.enter_context(tc.tile_pool(name="consts", bufs=1))

            ident_f = consts.tile([128, 128], F32, name="ident_f")
            make_identity(nc, ident_f)
            ident_b = consts.tile([128, 128], BF16, name="ident_b")
            nc.vector.tensor_copy(out=ident_b, in_=ident_f)

            eps_t = consts.tile([128, 1], F32, name="eps_t")
            nc.vector.memset(eps_t, 1e-5)
            ones1 = consts.tile([128, 1], F32, name="ones1")
            nc.vector.memset(ones1, 1.0)
            ones1b = consts.tile([128, 1], BF16, name="ones1b")
            nc.vector.memset(ones1b, 1.0)
            ones16b = consts.tile([128, 16], BF16, name="ones16b")
            nc.vector.memset(ones16b, 1.0)

            lam_t = consts.tile([1, 1], F32, name="lam_t")
            nc.sync.dma_start(out=lam_t, in_=lam.ap())
            sg_t = consts.tile([1, 1], F32, name="sg_t")
            nc.scalar.activation(out=sg_t, in_=lam_t, func=AF.Sigmoid)
            sig_bc = consts.tile([128, 1], F32, name="sig_bc")
            nc.gpsimd.partition_broadcast(sig_bc, sg_t)
            oms_bc = consts.tile([128, 1], F32, name="oms_bc")
            nc.vector.tensor_sub(oms_bc, ones1, sig_bc)

            # packed bias columns (one contiguous DMA)
            bcols_t = consts.tile([128, 50], F32, name="bcols_t")
            nc.gpsimd.dma_start(out=bcols_t, in_=bcols.ap())
            bq_c = bcols_t[:, 0:8]
            bk_c = bcols_t[:, 8:16]
            b1_c = bcols_t[:, 16:48]
            bqp_c = bcols_t[0:64, 48:49]
            bkp_c = bcols_t[0:64, 49:50]

            # bisect state
            bis = est.enter_context(tc.tile_pool(name="bis", bufs=1))
            lo = bis.tile([128, NQT], F32, name="lo")
            hi = bis.tile([128, NQT], F32, name="hi")
            mid = bis.tile([128, NQT], F32, name="mid")
            cnts = bis.tile([128, NQT], F32, name="cnts")
            pred = bis.tile([128, NQT], mybir.dt.uint32, name="pred")
            rs_sp = bis.tile([128, NQT], F32, name="rs_sp")
            rcp_sp = bis.tile([128, NQT], F32, name="rcp_sp")

            # long-lived activations
            sp_stack = contextlib.ExitStack()
            sp_pool = sp_stack.enter_context(
                tc.tile_pool(name="sp_pool", bufs=1))
            psp = sp_pool.tile([128, NQT, S], BF16, name="psp")
            kspT = sp_pool.tile([64, S], F32R, name="kspT")
            qspT = sp_pool.tile([64, SQ], F32R, name="qspT")

            av_stack = contextlib.ExitStack()
            av_pool = av_stack.enter_context(
                tc.tile_pool(name="av_pool", bufs=1))
            Vaug = av_pool.tile([128, NTOK, H * (DH + 1)], BF16, name="Vaug")
            Vsp = av_pool.tile([128, NTOK, D], BF16, name="Vsp")
            ctxT = av_pool.tile([128, KC, SQ], BF16, name="ctxT")

            Vaug_h = Vaug.rearrange("p t (h c) -> p t h c", c=DH + 1)
            for t in range(NTOK):
                nc.vector.tensor_copy(out=Vaug_h[:, t, :, DH:DH + 1],
                                      in_=ones16b)

            xot_stack = contextlib.ExitStack()
            xot_pool = xot_stack.enter_context(tc.tile_pool(name="xot_pool",
                                                            bufs=1))
            xot = xot_pool.tile([128, NQT, D], F32, name="xot")

            # out_proj weights (DMA issued later, after the startup crunch)
            wo_stack = contextlib.ExitStack()
            wo_pool = wo_stack.enter_context(
                tc.tile_pool(name="wo_pool", bufs=1))
            woT_s = wo_pool.tile([128, KC, D], BF16, name="woT_s")

            # =========== phase 0/1: input loads, sparse + v projections ====
            xbf_stack = contextlib.ExitStack()
            xbf_pool = xbf_stack.enter_context(
                tc.tile_pool(name="xbf_pool", bufs=1))
            xbf = xbf_pool.tile([128, KC, S], BF16, name="xbf")
            x_stack = contextlib.ExitStack()
            xt_pool = x_stack.enter_context(tc.tile_pool(name="xt_pool",
                                                         bufs=1))
            xTt = xt_pool.tile([128, KC, S], F32R, name="xTt")

            # early broadcast rows: bv, bvp, sig*bo
            early_stack = contextlib.ExitStack()
            early_bc = early_stack.enter_context(
                tc.tile_pool(name="early_bc", bufs=1))
            with contextlib.ExitStack() as brow_stack:
                brow_pool = brow_stack.enter_context(
                    tc.tile_pool(name="brow_pool", bufs=1))
                brow_t = brow_pool.tile([1, 3 * D], F32, name="brow_t")
                nc.gpsimd.dma_start(out=brow_t, in_=brow.ap()[:, 0:3 * D])
                bv_bc = early_bc.tile([128, D], F32, name="bv_bc")
                nc.gpsimd.partition_broadcast(bv_bc, brow_t[:, 0 * D:1 * D])
                bvp_bc = early_bc.tile([128, D], F32, name="bvp_bc")
                nc.gpsimd.partition_broadcast(bvp_bc, brow_t[:, 1 * D:2 * D])
                bo_sig = early_bc.tile([128, D], F32, name="bo_sig")
                nc.gpsimd.partition_broadcast(bo_sig, brow_t[:, 2 * D:3 * D])
                nc.vector.tensor_scalar_mul(bo_sig, bo_sig, sig_bc)

            with contextlib.ExitStack() as ph0:
                wsp_pool = ph0.enter_context(
                    tc.tile_pool(name="wsp_pool", bufs=1))
                ps_proj = ph0.enter_context(
                    tc.tile_pool(name="ps_proj", bufs=3, space="PSUM"))

                qkpt = wsp_pool.tile([128, KC, 2 * R], F32R, name="qkpt")
                nc.sync.dma_start(out=qkpt, in_=qkp_r)
                qpt = qkpt[:, :, 0:R]
                kpt = qkpt[:, :, R:2 * R]
                for kc in range(KC):
                    eng = nc.scalar if kc % 2 == 0 else nc.sync
                    eng.dma_start(out=xTt[:, kc, :], in_=xT_r[:, kc, :])

                # ---- sparse projections + scores (f32r) ----
                with nc.named_scope("p0_ksp_qsp"):
                    for nh in range(2):
                        ps = ps_proj.tile([128, 512], F32, name="ps",
                                          tag="ps")
                        for kc in range(KC):
                            nc.tensor.matmul(
                                ps[0:64, :], kpt[:, kc, :],
                                xTt[:, kc, nh * 512:nh * 512 + 512],
                                start=(kc == 0), stop=(kc == KC - 1))
                        nc.scalar.activation(
                            out=kspT[:, nh * 512:nh * 512 + 512],
                            in_=ps[0:64, :], func=AF.Identity, bias=bkp_c,
                            scale=1.0)
                    ps = ps_proj.tile([128, 512], F32, name="ps", tag="ps")
                    for kc in range(KC):
                        nc.tensor.matmul(ps[0:64, :], qpt[:, kc, :],
                                         xTt[:, kc, 0:SQ],
                                         start=(kc == 0), stop=(kc == KC - 1))
                    nc.scalar.activation(out=qspT, in_=ps[0:64, :],
                                         func=AF.Identity, bias=bqp_c,
                                         scale=1.0)

                with nc.named_scope("p2_ssp"):
                    for qt in range(NQT):
                        for nh in range(2):
                            ps = ps_proj.tile([128, 512], F32, name="ps",
                                              tag="ps")
                            nc.tensor.matmul(
                                ps, qspT[:, qt * 128:qt * 128 + 128],
                                kspT[:, nh * 512:nh * 512 + 512],
                                start=True, stop=True)
                            nc.scalar.activation(
                                out=psp[:, qt, nh * 512:nh * 512 + 512],
                                in_=ps, func=AF.Exp, scale=INV_SQRT)

                # cast xT to bf16 for the trunk projections
                with nc.named_scope("p0_cast"):
                    for kc in range(KC):
                        nc.vector.tensor_copy(out=xbf[:, kc, :],
                                              in_=xTt[:, kc, :])

                # own-token residual (+ sig*bo)
                for qt in range(NQT):
                    nc.scalar.dma_start(
                        out=xot[:, qt, :],
                        in_=x_own.ap()[qt * 128:qt * 128 + 128, :])
                    nc.gpsimd.tensor_add(xot[:, qt, :], xot[:, qt, :],
                                         bo_sig)

            # ---- v/vsp x-stationary projections -> token-major ----
            with contextlib.ExitStack() as ph4:
                # reopened weight pool (wv_s, wvp_s still live via av? no --
                # keep them in this scope)
                wv_pool2 = ph4.enter_context(
                    tc.tile_pool(name="wv_pool2", bufs=1))
                wv_s = wv_pool2.tile([128, KC, D], BF16, name="wv_s2")
                nc.sync.dma_start(out=wv_s, in_=wqkvT_r[:, :, 2 * D:3 * D])
                wvp_s = wv_pool2.tile([128, KC, D], BF16, name="wvp_s2")
                nc.sync.dma_start(out=wvp_s, in_=vpT_r)
                ps_v = ph4.enter_context(
                    tc.tile_pool(name="ps_v", bufs=8, space="PSUM"))
                with nc.named_scope("p4_v"):
                    for t in range(NTOK):
                        pva0 = ps_v.tile([128, 512], F32, name="pv", tag="pv")
                        pva1 = ps_v.tile([128, 512], F32, name="pv", tag="pv")
                        pvs0 = ps_v.tile([128, 512], F32, name="pv", tag="pv")
                        pvs1 = ps_v.tile([128, 512], F32, name="pv", tag="pv")
                        for kc in range(KC):
                            xck = xbf[:, kc, t * 128:t * 128 + 128]
                            st = (kc == 0)
                            sp = (kc == KC - 1)
                            nc.tensor.matmul(pva0, xck, wv_s[:, kc, 0:512],
                                             start=st, stop=sp)
                            nc.tensor.matmul(pva1, xck, wv_s[:, kc, 512:1024],
                                             start=st, stop=sp)
                            nc.tensor.matmul(pvs0, xck, wvp_s[:, kc, 0:512],
                                             start=st, stop=sp)
                            nc.tensor.matmul(pvs1, xck, wvp_s[:, kc, 512:1024],
                                             start=st, stop=sp)
                        nc.vector.tensor_add(
                            Vaug_h[:, t, 0:8, 0:DH], pva0, bv_bc[:, 0:512])
                        nc.vector.tensor_add(
                            Vaug_h[:, t, 8:16, 0:DH], pva1, bv_bc[:, 512:1024])
                        nc.vector.tensor_add(
                            Vsp[:, t, 0:512], pvs0, bvp_bc[:, 0:512])
                        nc.vector.tensor_add(
                            Vsp[:, t, 512:1024], pvs1, bvp_bc[:, 512:1024])
            early_stack.close()   # free bv_bc, bvp_bc, bo_sig

            # ---- top-k threshold bisection (DVE, bf16 scans) ----
            scr_stack = contextlib.ExitStack()
            scr_pool = scr_stack.enter_context(
                tc.tile_pool(name="scr", bufs=2))
            with nc.named_scope("p3_bisect"):
                nc.vector.memset(lo, 0.0)
                nc.vector.memset(hi, 16.0)
                for it in range(BISECT_ITERS):
                    nc.vector.tensor_add(mid, lo, hi)
                    nc.vector.tensor_scalar_mul(mid, mid, 0.5)
                    for qt in range(NQT):
                        scr = scr_pool.tile([128, S], BF16, name="scr",
                                            tag="scr")
                        nc.vector.scalar_tensor_tensor(
                            out=scr, in0=psp[:, qt, :],
                            scalar=mid[:, qt:qt + 1],
                            in1=ones1b.to_broadcast([128, S]),
                            op0=OP.is_ge, op1=OP.mult,
                            accum_out=cnts[:, qt:qt + 1])
                    nc.vector.tensor_scalar(out=pred, in0=cnts,
                                            scalar1=float(KK),
                                            scalar2=None, op0=OP.is_ge)
                    nc.vector.copy_predicated(lo, pred, mid)
                    nc.vector.tensor_scalar(out=pred, in0=cnts,
                                            scalar1=float(KK),
                                            scalar2=None, op0=OP.is_lt)
                    nc.vector.copy_predicated(hi, pred, mid)
                for qt in range(NQT):
                    nc.vector.scalar_tensor_tensor(
                        out=psp[:, qt, :], in0=psp[:, qt, :],
                        scalar=lo[:, qt:qt + 1],
                        in1=psp[:, qt, :], op0=OP.is_ge, op1=OP.mult,
                        accum_out=rs_sp[:, qt:qt + 1])
                nc.vector.tensor_scalar(out=rs_sp, in0=rs_sp,
                                        scalar1=1e-9, scalar2=None,
                                        op0=OP.add)
                nc.vector.reciprocal(rcp_sp, rs_sp)
                nc.vector.tensor_scalar_mul(rcp_sp, rcp_sp, oms_bc)

            scr_stack.close()
            x_stack.close()   # free xTt (f32)

            # out_proj weights resident (used in p6)
            nc.sync.dma_start(out=woT_s, in_=woT_r)

            # ======== phase 5: k/q projections + pipelined attention =======
            kq_stack = contextlib.ExitStack()
            kq_pool = kq_stack.enter_context(
                tc.tile_pool(name="kq_pool", bufs=1))
            kT = kq_pool.tile([128, KC, S], BF16, name="kT")
            qT = kq_pool.tile([128, KC, SQ], BF16, name="qT")
            with contextlib.ExitStack() as ph5:
                wstr = ph5.enter_context(tc.tile_pool(name="wstr", bufs=3))
                pt_pool = ph5.enter_context(
                    tc.tile_pool(name="pt_pool", bufs=16))
                rc_pool = ph5.enter_context(
                    tc.tile_pool(name="rc_pool", bufs=2))
                ps_kq = ph5.enter_context(
                    tc.tile_pool(name="ps_kq", bufs=3, space="PSUM"))
                ps_attn = ph5.enter_context(
                    tc.tile_pool(name="ps_attn", bufs=3, space="PSUM"))
                ps_ctx = ph5.enter_context(
                    tc.tile_pool(name="ps_ctx", bufs=2, space="PSUM"))

                pts = {}

                def proj_piece(ft):
                    wk = wstr.tile([128, KC, 128], BF16, name="wk", tag="wk")
                    nc.sync.dma_start(
                        out=wk, in_=wqkvT_r[:, :, D + ft * 128:D + ft * 128 + 128])
                    wq = wstr.tile([128, KC, 128], BF16, name="wq", tag="wq")
                    nc.sync.dma_start(
                        out=wq, in_=wqkvT_r[:, :, ft * 128:ft * 128 + 128])
                    for nh in range(2):
                        ps = ps_kq.tile([128, 512], F32, name="pkq",
                                        tag="pkq")
                        for kc in range(KC):
                            nc.tensor.matmul(
                                ps, wk[:, kc, :],
                                xbf[:, kc, nh * 512:nh * 512 + 512],
                                start=(kc == 0), stop=(kc == KC - 1))
                        nc.vector.tensor_scalar(
                            out=kT[:, ft, nh * 512:nh * 512 + 512],
                            in0=ps, scalar1=bk_c[:, ft:ft + 1],
                            scalar2=None, op0=OP.add)
                    ps = ps_kq.tile([128, 512], F32, name="pkq", tag="pkq")
                    for kc in range(KC):
                        nc.tensor.matmul(ps, wq[:, kc, :], xbf[:, kc, 0:SQ],
                                         start=(kc == 0), stop=(kc == KC - 1))
                    nc.vector.tensor_scalar(
                        out=qT[:, ft, :], in0=ps,
                        scalar1=bq_c[:, ft:ft + 1], scalar2=None, op0=OP.add)

                def scores(hh):
                    po = 64 * (hh % 2)
                    ft = hh // 2
                    tiles = []
                    for t in range(NTOK):
                        ps = ps_attn.tile([128, 512], F32, name="ps_s",
                                          tag="ps_s")
                        nc.tensor.matmul(
                            ps, kT[po:po + 64, ft, t * 128:t * 128 + 128],
                            qT[po:po + 64, ft, :], start=True, stop=True)
                        pt = pt_pool.tile([128, 512], BF16, name="pT",
                                          tag="pT")
                        nc.scalar.activation(out=pt, in_=ps, func=AF.Exp,
                                             scale=INV_SQRT)
                        tiles.append(pt)
                    pts[hh] = tiles

                def ctx(hh):
                    po = 64 * (hh % 2)
                    ft = hh // 2
                    pctx = ps_ctx.tile([128, 512], F32, name="ps_c",
                                       tag="ps_c")
                    for t in range(NTOK):
                        nc.tensor.matmul(
                            pctx[0:65, :], Vaug_h[:, t, hh, 0:DH + 1],
                            pts[hh][t], start=(t == 0), stop=(t == NTOK - 1))
                    rsr = rc_pool.tile([1, 512], F32, name="rsr", tag="rsr")
                    nc.vector.tensor_copy(out=rsr, in_=pctx[64:65, :])
                    rch = rc_pool.tile([1, 512], F32, name="rch", tag="rch")
                    nc.vector.reciprocal_approx_fast(out=rch, in_=rsr)
                    rb = rc_pool.tile([64, 512], F32, name="rb", tag="rb")
                    nc.gpsimd.partition_broadcast(rb, rch)
                    nc.vector.tensor_mul(out=ctxT[po:po + 64, ft, :],
                                         in0=pctx[0:64, :], in1=rb)
                    del pts[hh]

                with nc.named_scope("p5_kq_attn"):
                    proj_piece(0)
                    proj_piece(1)
                    for hh in range(H):
                        if hh % 2 == 0 and hh // 2 + 2 < KC:
                            proj_piece(hh // 2 + 2)
                        scores(hh)
                        if hh >= 1:
                            ctx(hh - 1)
                    ctx(H - 1)

            kq_stack.close()    # free kT, qT
            xbf_stack.close()   # free xbf

            # ========= phase 6: outproj + spmm + fuse + LN1 (qt-outer) =====
            # late broadcast rows: b12, g1, g2, be2 (right-side stack)
            late_bc = est.enter_context(
                tc.tile_pool(name="late_bc", bufs=1, side="right"))
            with contextlib.ExitStack() as brow_stack:
                brow_pool = brow_stack.enter_context(
                    tc.tile_pool(name="brow_pool2", bufs=1, side="right"))
                brow_t = brow_pool.tile([1, 4 * D], F32, name="brow_t2")
                nc.gpsimd.dma_start(out=brow_t,
                                    in_=brow.ap()[:, 3 * D:7 * D])
                b12_bc = late_bc.tile([128, D], F32, name="b12_bc")
                nc.gpsimd.partition_broadcast(b12_bc, brow_t[:, 0:D])
                g1_bc = late_bc.tile([128, D], F32, name="g1_bc")
                nc.gpsimd.partition_broadcast(g1_bc, brow_t[:, D:2 * D])
                g2_bc = late_bc.tile([128, D], F32, name="g2_bc")
                nc.gpsimd.partition_broadcast(g2_bc, brow_t[:, 2 * D:3 * D])
                be2_bc = late_bc.tile([128, D], F32, name="be2_bc")
                nc.gpsimd.partition_broadcast(be2_bc, brow_t[:, 3 * D:4 * D])

            fse = est.enter_context(tc.tile_pool(name="fse", bufs=1,
                                                 side="right"))
            x1 = fse.tile([128, NQT, D], F32, name="x1")
            mv2 = fse.tile([128, NQT, 2], F32, name="mv2")
            stats = fse.tile([128, NQT, 2, 6], F32, name="stats")
            sd = fse.tile([128, NQT], F32, name="sd")
            rstd = fse.tile([128, NQT], F32, name="rstd")

            xln_stack = contextlib.ExitStack()
            xlnT_pool = xln_stack.enter_context(
                tc.tile_pool(name="xlnT_pool", bufs=1, side="right"))
            xlnT = xlnT_pool.tile([128, KC, SQ], BF16, name="xlnT")
            xbf1_stack = contextlib.ExitStack()
            xbf1_pool = xbf1_stack.enter_context(
                tc.tile_pool(name="xbf1_pool", bufs=1, side="right"))
            xbf1 = xbf1_pool.tile([128, NQT, D], BF16, name="xbf1")

            def ln_stats(src_ap, qt):
                for half in range(2):
                    nc.vector.bn_stats(
                        out=stats[:, qt, half, :],
                        in_=src_ap[:, half * 512:half * 512 + 512])
                nc.vector.bn_aggr(out=mv2[:, qt, :], in_=stats[:, qt])
                nc.scalar.activation(out=sd[:, qt:qt + 1],
                                     in_=mv2[:, qt, 1:2], func=AF.Sqrt,
                                     bias=eps_t, scale=1.0)
                nc.vector.reciprocal(rstd[:, qt:qt + 1], sd[:, qt:qt + 1])

            with contextlib.ExitStack() as ph6:
                pm_pool = ph6.enter_context(tc.tile_pool(name="pm_pool",
                                                         bufs=2))
                ps_o = ph6.enter_context(
                    tc.tile_pool(name="ps_o", bufs=4, space="PSUM"))
                ps_sp = ph6.enter_context(
                    tc.tile_pool(name="ps_sp", bufs=2, space="PSUM"))
                ps_tr = ph6.enter_context(
                    tc.tile_pool(name="ps_tr", bufs=2, space="PSUM"))
                with nc.named_scope("p6_fuse"):
                    for qt in range(NQT):
                        qc = slice(qt * 128, qt * 128 + 128)
                        # out_proj (2 halves, ctxT-stationary)
                        po0 = ps_o.tile([128, 512], F32, name="po", tag="po")
                        po1 = ps_o.tile([128, 512], F32, name="po", tag="po")
                        for kc in range(KC):
                            st, sp = (kc == 0), (kc == KC - 1)
                            nc.tensor.matmul(po0, ctxT[:, kc, qc],
                                             woT_s[:, kc, 0:512],
                                             start=st, stop=sp)
                            nc.tensor.matmul(po1, ctxT[:, kc, qc],
                                             woT_s[:, kc, 512:1024],
                                             start=st, stop=sp)
                        # masked-p transposes for this qt
                        pmt = pm_pool.tile([128, NTOK, 128], BF16, name="pmt",
                                           tag="pmt")
                        for t in range(NTOK):
                            pst = ps_tr.tile([128, 128], BF16, name="pst",
                                             tag="pst")
                            nc.tensor.transpose(
                                pst, psp[:, qt, t * 128:t * 128 + 128],
                                ident_b)
                            nc.vector.tensor_copy(out=pmt[:, t, :], in_=pst)
                        # spmm (2 halves)
                        sp0 = ps_sp.tile([128, 512], F32, name="psp2",
                                         tag="psp2")
                        sp1 = ps_sp.tile([128, 512], F32, name="psp2",
                                         tag="psp2")
                        for t in range(NTOK):
                            st, spl = (t == 0), (t == NTOK - 1)
                            nc.tensor.matmul(sp0, pmt[:, t, :],
                                             Vsp[:, t, 0:512],
                                             start=st, stop=spl)
                            nc.tensor.matmul(sp1, pmt[:, t, :],
                                             Vsp[:, t, 512:1024],
                                             start=st, stop=spl)
                        # fuse on DVE: x1 = sig*dense + rcp*spmm + xot
                        xq = x1[:, qt, :]
                        nc.vector.tensor_scalar(
                            out=xq[:, 0:512], in0=po0, scalar1=sig_bc,
                            scalar2=None, op0=OP.mult)
                        nc.vector.tensor_scalar(
                            out=xq[:, 512:1024], in0=po1, scalar1=sig_bc,
                            scalar2=None, op0=OP.mult)
                        nc.vector.tensor_add(xq, xq, xot[:, qt, :])
                        nc.vector.scalar_tensor_tensor(
                            out=xq[:, 0:512], in0=sp0,
                            scalar=rcp_sp[:, qt:qt + 1],
                            in1=xq[:, 0:512], op0=OP.mult, op1=OP.add)
                        nc.vector.scalar_tensor_tensor(
                            out=xq[:, 512:1024], in0=sp1,
                            scalar=rcp_sp[:, qt:qt + 1],
                            in1=xq[:, 512:1024], op0=OP.mult, op1=OP.add)
                        # LN1 (keep x1 raw f32 for the ff2 residual)
                        ln_stats(xq, qt)
                        nc.vector.tensor_scalar(
                            out=xbf1[:, qt, :], in0=xq,
                            scalar1=mv2[:, qt, 0:1],
                            scalar2=rstd[:, qt:qt + 1],
                            op0=OP.subtract, op1=OP.mult)
                        # transpose normalized qt block for ff1
                        for fc in range(KC):
                            pst = ps_tr.tile([128, 128], BF16, name="pst",
                                             tag="pst")
                            nc.tensor.transpose(
                                pst, xbf1[:, qt, fc * 128:fc * 128 + 128],
                                ident_b)
                            nc.vector.tensor_copy(out=xlnT[:, fc, qc],
                                                  in_=pst)

            xbf1_stack.close()
            wo_stack.close()
            xot_stack.close()
            av_stack.close()   # free Vaug, Vsp, ctxT
            sp_stack.close()   # free psp, kspT, qspT

            # f2T resident for qt-outer ff2 (DMA hides under ff1)
            f2_stack = contextlib.ExitStack()
            f2_pool = f2_stack.enter_context(
                tc.tile_pool(name="f2_pool", bufs=1))
            f2_s = f2_pool.tile([128, FC, D], BF16, name="f2_s")
            nc.sync.dma_start(out=f2_s, in_=f2T_r)

            # xg = xhat*g1 + (be1+b2), computed on DVE during ff1
            xg = fse.tile([128, NQT, D], F32, name="xg")

            # ============ ff1 + relu ============
            h1_stack = contextlib.ExitStack()
            h1_pool = h1_stack.enter_context(
                tc.tile_pool(name="h1_pool", bufs=1))
            h1T = h1_pool.tile([128, FC, SQ], BF16, name="h1T")
            with contextlib.ExitStack() as ph9:
                w3str = ph9.enter_context(tc.tile_pool(name="w3str", bufs=2))
                ps_f1 = ph9.enter_context(
                    tc.tile_pool(name="ps_f1", bufs=4, space="PSUM"))
                with nc.named_scope("p9_ff1"):
                    for jj in range(16):
                        wt = w3str.tile([128, KC, 256], BF16, name="w1t",
                                        tag="w3")
                        f0 = jj * 256
                        eng = nc.scalar if jj % 2 == 0 else nc.sync
                        eng.dma_start(out=wt, in_=f1T_r[:, :, f0:f0 + 256])
                        for fi in range(2):
                            dft = jj * 2 + fi
                            ps = ps_f1.tile([128, 512], F32, name="ps_f",
                                            tag="ps_f")
                            for kc in range(KC):
                                nc.tensor.matmul(
                                    ps, wt[:, kc, fi * 128:fi * 128 + 128],
                                    xlnT[:, kc, :],
                                    start=(kc == 0), stop=(kc == KC - 1))
                            nc.scalar.activation(
                                out=h1T[:, dft, :], in_=ps, func=AF.Relu,
                                bias=b1_c[:, dft:dft + 1], scale=1.0)
                        if jj < 2 * NQT and jj % 2 == 1:
                            # xg for qt = jj//2, hidden under ff1
                            qt = jj // 2
                            nc.vector.tensor_scalar(
                                out=xg[:, qt, :], in0=x1[:, qt, :],
                                scalar1=mv2[:, qt, 0:1],
                                scalar2=rstd[:, qt:qt + 1],
                                op0=OP.subtract, op1=OP.mult)
                            nc.vector.tensor_mul(xg[:, qt, :], xg[:, qt, :],
                                                 g1_bc)
                            nc.vector.tensor_add(xg[:, qt, :], xg[:, qt, :],
                                                 b12_bc)
            xln_stack.close()

            # ============ ff2 (qt-outer) + residual + LN2 + out ============
            with contextlib.ExitStack() as ph10:
                ps_f2 = ph10.enter_context(
                    tc.tile_pool(name="ps_f2", bufs=4, space="PSUM"))
                ot_pool = ph10.enter_context(
                    tc.tile_pool(name="ot_pool", bufs=2))
                with nc.named_scope("p10_ff2"):
                    for qt in range(NQT):
                        qc = slice(qt * 128, qt * 128 + 128)
                        pg0 = ps_f2.tile([128, 512], F32, name="pg", tag="pg")
                        pg1 = ps_f2.tile([128, 512], F32, name="pg", tag="pg")
                        for kc in range(FC):
                            st, sp = (kc == 0), (kc == FC - 1)
                            nc.tensor.matmul(pg0, h1T[:, kc, qc],
                                             f2_s[:, kc, 0:512],
                                             start=st, stop=sp)
                            nc.tensor.matmul(pg1, h1T[:, kc, qc],
                                             f2_s[:, kc, 512:1024],
                                             start=st, stop=sp)
                        x2 = x1[:, qt, :]
                        nc.vector.tensor_add(x2[:, 0:512], pg0,
                                             xg[:, qt, 0:512])
                        nc.vector.tensor_add(x2[:, 512:1024], pg1,
                                             xg[:, qt, 512:1024])
                        ln_stats(x2, qt)
                        ot = ot_pool.tile([128, D], F32, name="out_t",
                                          tag="out_t")
                        nc.vector.tensor_scalar(
                            out=ot, in0=x2, scalar1=mv2[:, qt, 0:1],
                            scalar2=rstd[:, qt:qt + 1],
                            op0=OP.subtract, op1=OP.mult)
                        nc.vector.tensor_mul(ot, ot, g2_bc)
                        nc.vector.tensor_add(ot, ot, be2_bc)
                        nc.scalar.dma_start(
                            out=out.ap()[qt * 128:qt * 128 + 128, :], in_=ot)
            h1_stack.close()
            f2_stack.close()

    nc.compile()
    return nc


def _prep_inputs(src, in_proj_w, in_proj_b, out_proj_w, out_proj_b,
                 Qp_w, Qp_b, Kp_w, Kp_b, Vp_w, Vp_b, lam,
                 ff1_w, ff1_b, ff2_w, ff2_b, ln1_g, ln1_b, ln2_g, ln2_b):
    import ml_dtypes
    f = np.float32
    A = lambda x: np.ascontiguousarray(x, dtype=f)
    AB = lambda x: np.ascontiguousarray(np.asarray(x, dtype=f),
                                        dtype=ml_dtypes.bfloat16)
    in_proj_w = np.asarray(in_proj_w, dtype=f)
    ff1_w = np.asarray(ff1_w, dtype=f)
    ln1_g = np.asarray(ln1_g, dtype=f)
    ln1_b = np.asarray(ln1_b, dtype=f)
    # fold ln1 gamma into ff1 weights, ln1 beta into ff1 bias
    f1_eff = ff1_w * ln1_g[None, :]            # [DFF, D]
    b1_eff = np.asarray(ff1_b, dtype=f) + ff1_w @ ln1_b

    # packed per-partition bias columns [128, 50]
    bcols = np.zeros((128, 50), dtype=f)
    qk_bias = np.asarray(in_proj_b, dtype=f)
    bcols[:, 0:8] = qk_bias[0:D].reshape(8, 128).T
    bcols[:, 8:16] = qk_bias[D:2 * D].reshape(8, 128).T
    bcols[:, 16:48] = b1_eff.reshape(32, 128).T
    bcols[0:64, 48] = np.asarray(Qp_b, dtype=f)
    bcols[0:64, 49] = np.asarray(Kp_b, dtype=f)

    # packed broadcast rows [1, 7D]: bv, bvp, bo, b12, g1, g2, be2
    brow = np.concatenate([
        qk_bias[2 * D:3 * D],
        np.asarray(Vp_b, dtype=f),
        np.asarray(out_proj_b, dtype=f),
        ln1_b + np.asarray(ff2_b, dtype=f),
        ln1_g,
        np.asarray(ln2_g, dtype=f),
        np.asarray(ln2_b, dtype=f),
    ]).reshape(1, 7 * D)

    # qkp packed partition-major: [128, kc, 2R] contiguous
    qkpT = np.concatenate([np.asarray(Qp_w).T, np.asarray(Kp_w).T], axis=1)
    qkp_pm = np.ascontiguousarray(
        qkpT.reshape(KC, 128, 2 * R).transpose(1, 0, 2).reshape(128, -1),
        dtype=f)

    shared = {
        "wqkvT": AB(in_proj_w.T),
        "woT": AB(np.asarray(out_proj_w).T),
        "vpT": AB(np.asarray(Vp_w).T),
        "qkp_pm": qkp_pm,
        "f1T": AB(f1_eff.T),
        "f2T": AB(np.asarray(ff2_w).T),
        "bcols": bcols,
        "brow": A(brow),
        "lam": A(np.asarray(lam)).reshape(1, 1),
    }
    in_maps = []
    for core in range(8):
        b, h = core // 2, core % 2
        srcb = np.asarray(src[b])
        xTb = srcb.T
        if h == 1:
            # own-query columns first (key order is irrelevant to attention)
            xTb = np.concatenate([xTb[:, SQ:], xTb[:, :SQ]], axis=1)
        m = dict(shared)
        m["xT"] = A(xTb)
        m["x_own"] = A(srcb[h * SQ:(h + 1) * SQ, :])
        in_maps.append(m)
    return in_maps


def _run(inputs, trace=False):
    if "nc" not in _cached:
        _cached["nc"] = _build()
    nc = _cached["nc"]
    in_maps = _prep_inputs(**inputs)
    res = run_bass_kernel_spmd(nc, in_maps, core_ids=list(range(8)),
                               trace=trace)
    out = np.empty((B, S, D), np.float32)
    for core in range(8):
        b, h = core // 2, core % 2
        out[b, h * SQ:(h + 1) * SQ, :] = res.results[core]["out"]
    return out, res


def kernel(**inputs) -> np.ndarray:
    out, _ = _run(inputs, trace=False)
    return out


# revision 21
# speedup vs baseline: 1.0929x; 1.0833x over previous
"""Trainium2 Bass kernel for the EnhancedEncoderLayer (dense MHA + low-rank
top-k sparse attention + FFN, two layernorms).

Sharding: 8 cores = (batch b in 0..3) x (query-half h in {0,1}). Each core
computes output rows [b, h*512:(h+1)*512, :]. K/V-side projections are
computed redundantly per batch pair (no cross-core communication).

The host permutes src[b].T columns so each core's own query tokens are
columns 0..511 (attention contracts over all keys, so key order is
irrelevant); this keeps the SPMD program identical across cores.

v2 design notes:
- v/vsp projections are x-stationary (lhsT = xT chunk), producing
  token-major Vaug/Vsp directly -- no PE transposes, no ACT copies.
- dense attention is software-pipelined: scores(h) / proj filler /
  ctx(h-1), so ACT exp latency never stalls the in-order PE queue.
- the sparse path runs in bf16: exp writes bf16 psp, the top-k threshold
  bisection scans at 2x DVE rate with 18 iterations, spmm is bf16.
- outproj+spmm+fuse+LN1 run qt-outer so LN1 overlaps matmuls; ff2 is
  qt-outer against an SBUF-resident f2T with LN2+output pipelined per qt.
- ln1 gamma/beta are folded into ff1 weights/bias host-side.
- all host tensors are laid out so every DMA is contiguous per partition.
"""
import sys
import os
import contextlib

for _p in ('/opt/trn_rl_repo',):
    if _p not in sys.path:
        sys.path.insert(0, _p)

import numpy as np
import concourse.bacc as bacc
import concourse.tile as tile
from concourse import mybir
from concourse.bass_utils import run_bass_kernel_spmd
from concourse.masks import make_identity

F32 = mybir.dt.float32
F32R = mybir.dt.float32r
BF16 = mybir.dt.bfloat16
AF = mybir.ActivationFunctionType
OP = mybir.AluOpType

B, S, D, H, R, DFF = 4, 1024, 1024, 16, 64, 4096
DH = D // H          # 64
SQ = S // 2          # 512 own queries per core
KK = max(1, int(S * 0.2))   # 204
KC = D // 128        # 8 contraction chunks over D
FC = DFF // 128      # 32 chunks over DFF
NQT = SQ // 128      # 4 query tiles
NTOK = S // 128      # 8 token tiles
BISECT_ITERS = 19
INV_SQRT = 0.125     # 1/sqrt(DH) == 1/sqrt(R)

_cached = {}


def _build():
    nc = bacc.Bacc()

    def din(name, shape, dt=F32):
        return nc.declare_dram_parameter(name, list(shape), dt, isOutput=False)

    xT = din("xT", [D, S])            # src[b].T, own-query columns first
    x_own = din("x_own", [SQ, D])     # own rows, token-major
    wqkvT = din("wqkvT", [D, 3 * D], BF16)
    woT = din("woT", [D, D], BF16)
    vpT = din("vpT", [D, D], BF16)
    qkp_pm = din("qkp_pm", [128, KC * 2 * R])   # partition-major packed
    f1T = din("f1T", [D, DFF], BF16)  # pre-scaled by ln1 gamma
    f2T = din("f2T", [DFF, D], BF16)
    # packed per-partition bias columns: [0:8]=q, [8:16]=k, [16:48]=b1_eff,
    # [48]=bqp (rows 0:64), [49]=bkp (rows 0:64)
    bcols = din("bcols", [128, 50])
    # packed broadcast rows: [bv, bvp, bo, b12, g1, g2, be2]
    brow = din("brow", [1, 7 * D])
    lam = din("lam", [1, 1])
    out = nc.declare_dram_parameter("out", [SQ, D], F32, isOutput=True)

    xT_r = xT.ap().bitcast(F32R).rearrange("(kc p) s -> p kc s", p=128)
    wqkvT_r = wqkvT.ap().rearrange("(kc p) f -> p kc f", p=128)
    woT_r = woT.ap().rearrange("(kc p) f -> p kc f", p=128)
    vpT_r = vpT.ap().rearrange("(kc p) f -> p kc f", p=128)
    qkp_r = qkp_pm.ap().bitcast(F32R).rearrange("p (kc f) -> p kc f", f=2 * R)
    f1T_r = f1T.ap().rearrange("(kc p) f -> p kc f", p=128)
    f2T_r = f2T.ap().rearrange("(kc p) f -> p kc f", p=128)

    with tile.TileContext(nc) as tc:
        est = contextlib.ExitStack()
        with est:
            # ---------------- constants ----------------
            consts = est.enter_context(tc.tile_pool(name="consts", bufs=1))

            ident_f = consts.tile([128, 128], F32, name="ident_f")
            make_identity(nc, ident_f)
            ident_b = consts.tile([128, 128], BF16, name="ident_b")
            nc.vector.tensor_copy(out=ident_b, in_=ident_f)

            eps_t = consts.tile([128, 1], F32, name="eps_t")
            nc.vector.memset(eps_t, 1e-5)
            ones1 = consts.tile([128, 1], F32, name="ones1")
            nc.vector.memset(ones1, 1.0)
            ones1b = consts.tile([128, 1], BF16, name="ones1b")
            nc.vector.memset(ones1b, 1.0)
            ones16b = consts.tile([128, 16], BF16, name="ones16b")
            nc.vector.memset(ones16b, 1.0)

            lam_t = consts.tile([1, 1], F32, name="lam_t")
            nc.sync.dma_start(out=lam_t, in_=lam.ap())
            sg_t = consts.tile([1, 1], F32, name="sg_t")
            nc.scalar.activation(out=sg_t, in_=lam_t, func=AF.Sigmoid)
            sig_bc = consts.tile([128, 1], F32, name="sig_bc")
            nc.gpsimd.partition_broadcast(sig_bc, sg_t)
            oms_bc = consts.tile([128, 1], F32, name="oms_bc")
            nc.vector.tensor_sub(oms_bc, ones1, sig_bc)

            # packed bias columns (one contiguous DMA)
            bcols_t = consts.tile([128, 50], F32, name="bcols_t")
            nc.gpsimd.dma_start(out=bcols_t, in_=bcols.ap())
            bq_c = bcols_t[:, 0:8]
            bk_c = bcols_t[:, 8:16]
            b1_c = bcols_t[:, 16:48]
            bqp_c = bcols_t[0:64, 48:49]
            bkp_c = bcols_t[0:64, 49:50]

            # bisect state
            bis = est.enter_context(tc.tile_pool(name="bis", bufs=1))
            lo = bis.tile([128, NQT], F32, name="lo")
            hi = bis.tile([128, NQT], F32, name="hi")
            mid = bis.tile([128, NQT], F32, name="mid")
            cnts = bis.tile([128, NQT], F32, name="cnts")
            pred = bis.tile([128, NQT], mybir.dt.uint32, name="pred")
            rs_sp = bis.tile([128, NQT], F32, name="rs_sp")
            rcp_sp = bis.tile([128, NQT], F32, name="rcp_sp")

            # long-lived activations
            sp_stack = contextlib.ExitStack()
            sp_pool = sp_stack.enter_context(
                tc.tile_pool(name="sp_pool", bufs=1))
            psp = sp_pool.tile([128, NQT, S], BF16, name="psp")
            kspT = sp_pool.tile([64, S], F32R, name="kspT")
            qspT = sp_pool.tile([64, SQ], F32R, name="qspT")

            av_stack = contextlib.ExitStack()
            av_pool = av_stack.enter_context(
                tc.tile_pool(name="av_pool", bufs=1))
            Vaug = av_pool.tile([128, NTOK, H * (DH + 1)], BF16, name="Vaug")
            Vsp = av_pool.tile([128, NTOK, D], BF16, name="Vsp")
            ctxT = av_pool.tile([128, KC, SQ], BF16, name="ctxT")

            Vaug_h = Vaug.rearrange("p t (h c) -> p t h c", c=DH + 1)
            for t in range(NTOK):
                nc.vector.tensor_copy(out=Vaug_h[:, t, :, DH:DH + 1],
                                      in_=ones16b)

            xot_stack = contextlib.ExitStack()
            xot_pool = xot_stack.enter_context(tc.tile_pool(name="xot_pool",
                                                            bufs=1))
            xot = xot_pool.tile([128, NQT, D], F32, name="xot")

            # out_proj weights (DMA issued later, after the startup crunch)
            wo_stack = contextlib.ExitStack()
            wo_pool = wo_stack.enter_context(
                tc.tile_pool(name="wo_pool", bufs=1))
            woT_s = wo_pool.tile([128, KC, D], BF16, name="woT_s")

            # =========== phase 0/1: input loads, sparse + v projections ====
            xbf_stack = contextlib.ExitStack()
            xbf_pool = xbf_stack.enter_context(
                tc.tile_pool(name="xbf_pool", bufs=1))
            xbf = xbf_pool.tile([128, KC, S], BF16, name="xbf")
            x_stack = contextlib.ExitStack()
            xt_pool = x_stack.enter_context(tc.tile_pool(name="xt_pool",
                                                         bufs=1))
            xTt = xt_pool.tile([128, KC, S], F32R, name="xTt")

            # early broadcast rows: bv, bvp, sig*bo
            early_stack = contextlib.ExitStack()
            early_bc = early_stack.enter_context(
                tc.tile_pool(name="early_bc", bufs=1))
            with contextlib.ExitStack() as brow_stack:
                brow_pool = brow_stack.enter_context(
                    tc.tile_pool(name="brow_pool", bufs=1))
                brow_t = brow_pool.tile([1, 3 * D], F32, name="brow_t")
                nc.gpsimd.dma_start(out=brow_t, in_=brow.ap()[:, 0:3 * D])
                bv_bc = early_bc.tile([128, D], F32, name="bv_bc")
                nc.gpsimd.partition_broadcast(bv_bc, brow_t[:, 0 * D:1 * D])
                bvp_bc = early_bc.tile([128, D], F32, name="bvp_bc")
                nc.gpsimd.partition_broadcast(bvp_bc, brow_t[:, 1 * D:2 * D])
                bo_sig = early_bc.tile([128, D], F32, name="bo_sig")
                nc.gpsimd.partition_broadcast(bo_sig, brow_t[:, 2 * D:3 * D])
                nc.vector.tensor_scalar_mul(bo_sig, bo_sig, sig_bc)

            with contextlib.ExitStack() as ph0:
                wsp_pool = ph0.enter_context(
                    tc.tile_pool(name="wsp_pool", bufs=1))
                ps_proj = ph0.enter_context(
                    tc.tile_pool(name="ps_proj", bufs=3, space="PSUM"))

                qkpt = wsp_pool.tile([128, KC, 2 * R], F32R, name="qkpt")
                nc.sync.dma_start(out=qkpt, in_=qkp_r)
                qpt = qkpt[:, :, 0:R]
                kpt = qkpt[:, :, R:2 * R]
                for kc in range(KC):
                    eng = nc.scalar if kc % 2 == 0 else nc.sync
                    eng.dma_start(out=xTt[:, kc, :], in_=xT_r[:, kc, :])

                # ---- sparse projections + scores (f32r) ----
                with nc.named_scope("p0_ksp_qsp"):
                    for nh in range(2):
                        ps = ps_proj.tile([128, 512], F32, name="ps",
                                          tag="ps")
                        for kc in range(KC):
                            nc.tensor.matmul(
                                ps[0:64, :], kpt[:, kc, :],
                                xTt[:, kc, nh * 512:nh * 512 + 512],
                                start=(kc == 0), stop=(kc == KC - 1))
                        nc.scalar.activation(
                            out=kspT[:, nh * 512:nh * 512 + 512],
                            in_=ps[0:64, :], func=AF.Identity, bias=bkp_c,
                            scale=1.0)
                    ps = ps_proj.tile([128, 512], F32, name="ps", tag="ps")
                    for kc in range(KC):
                        nc.tensor.matmul(ps[0:64, :], qpt[:, kc, :],
                                         xTt[:, kc, 0:SQ],
                                         start=(kc == 0), stop=(kc == KC - 1))
                    nc.scalar.activation(out=qspT, in_=ps[0:64, :],
                                         func=AF.Identity, bias=bqp_c,
                                         scale=1.0)

                with nc.named_scope("p2_ssp"):
                    for qt in range(NQT):
                        for nh in range(2):
                            ps = ps_proj.tile([128, 512], F32, name="ps",
                                              tag="ps")
                            nc.tensor.matmul(
                                ps, qspT[:, qt * 128:qt * 128 + 128],
                                kspT[:, nh * 512:nh * 512 + 512],
                                start=True, stop=True)
                            nc.scalar.activation(
                                out=psp[:, qt, nh * 512:nh * 512 + 512],
                                in_=ps, func=AF.Exp, scale=INV_SQRT)

                # cast xT to bf16 for the trunk projections
                with nc.named_scope("p0_cast"):
                    for kc in range(KC):
                        nc.vector.tensor_copy(out=xbf[:, kc, :],
                                              in_=xTt[:, kc, :])

                # own-token residual (+ sig*bo)
                for qt in range(NQT):
                    nc.scalar.dma_start(
                        out=xot[:, qt, :],
                        in_=x_own.ap()[qt * 128:qt * 128 + 128, :])
                    nc.gpsimd.tensor_add(xot[:, qt, :], xot[:, qt, :],
                                         bo_sig)

            # ---- v/vsp x-stationary projections -> token-major ----
            with contextlib.ExitStack() as ph4:
                # reopened weight pool (wv_s, wvp_s still live via av? no --
                # keep them in this scope)
                wv_pool2 = ph4.enter_context(
                    tc.tile_pool(name="wv_pool2", bufs=1))
                wv_s = wv_pool2.tile([128, KC, D], BF16, name="wv_s2")
                nc.sync.dma_start(out=wv_s, in_=wqkvT_r[:, :, 2 * D:3 * D])
                wvp_s = wv_pool2.tile([128, KC, D], BF16, name="wvp_s2")
                nc.sync.dma_start(out=wvp_s, in_=vpT_r)
                ps_v = ph4.enter_context(
                    tc.tile_pool(name="ps_v", bufs=8, space="PSUM"))
                with nc.named_scope("p4_v"):
                    for t in range(NTOK):
                        pva0 = ps_v.tile([128, 512], F32, name="pv", tag="pv")
                        pva1 = ps_v.tile([128, 512], F32, name="pv", tag="pv")
                        pvs0 = ps_v.tile([128, 512], F32, name="pv", tag="pv")
                        pvs1 = ps_v.tile([128, 512], F32, name="pv", tag="pv")
                        for kc in range(KC):
                            xck = xbf[:, kc, t * 128:t * 128 + 128]
                            st = (kc == 0)
                            sp = (kc == KC - 1)
                            nc.tensor.matmul(pva0, xck, wv_s[:, kc, 0:512],
                                             start=st, stop=sp)
                            nc.tensor.matmul(pva1, xck, wv_s[:, kc, 512:1024],
                                             start=st, stop=sp)
                            nc.tensor.matmul(pvs0, xck, wvp_s[:, kc, 0:512],
                                             start=st, stop=sp)
                            nc.tensor.matmul(pvs1, xck, wvp_s[:, kc, 512:1024],
                                             start=st, stop=sp)
                        nc.vector.tensor_add(
                            Vaug_h[:, t, 0:8, 0:DH], pva0, bv_bc[:, 0:512])
                        nc.vector.tensor_add(
                            Vaug_h[:, t, 8:16, 0:DH], pva1, bv_bc[:, 512:1024])
                        nc.vector.tensor_add(
                            Vsp[:, t, 0:512], pvs0, bvp_bc[:, 0:512])
                        nc.vector.tensor_add(
                            Vsp[:, t, 512:1024], pvs1, bvp_bc[:, 512:1024])
            early_stack.close()   # free bv_bc, bvp_bc, bo_sig

            # out_proj weights resident (used in p6)
            nc.sync.dma_start(out=woT_s, in_=woT_r)

            # bisect scratch: lives until after the masking pass
            scr_stack = contextlib.ExitStack()
            scr_pool = scr_stack.enter_context(
                tc.tile_pool(name="scr", bufs=4))

            def bisect_iter():
                # one threshold-bisection step; qt 0-2 scan on DVE, qt 3 on
                # GpSimd (SBUF-only engine, otherwise idle here)
                nc.vector.tensor_add(mid, lo, hi)
                nc.vector.tensor_scalar_mul(mid, mid, 0.5)
                for qt in range(NQT):
                    scr = scr_pool.tile([128, S], BF16, name="scr",
                                        tag="scr")
                    nc.vector.tensor_scalar(
                        out=scr, in0=psp[:, qt, :],
                        scalar1=mid[:, qt:qt + 1], scalar2=1.0,
                        op0=OP.is_ge, op1=OP.mult,
                        accum_out=cnts[:, qt:qt + 1])
                nc.vector.tensor_scalar(out=pred, in0=cnts,
                                        scalar1=float(KK),
                                        scalar2=None, op0=OP.is_ge)
                nc.vector.copy_predicated(lo, pred, mid)
                nc.vector.tensor_scalar(out=pred, in0=cnts,
                                        scalar1=float(KK),
                                        scalar2=None, op0=OP.is_lt)
                nc.vector.copy_predicated(hi, pred, mid)

            # ======== phase 5: k/q projections + pipelined attention =======
            kq_stack = contextlib.ExitStack()
            kq_pool = kq_stack.enter_context(
                tc.tile_pool(name="kq_pool", bufs=1))
            kT = kq_pool.tile([128, KC, S], BF16, name="kT")
            qT = kq_pool.tile([128, KC, SQ], BF16, name="qT")
            with contextlib.ExitStack() as ph5:
                wstr = ph5.enter_context(tc.tile_pool(name="wstr", bufs=3))
                pt_pool = ph5.enter_context(
                    tc.tile_pool(name="pt_pool", bufs=16))
                rc_pool = ph5.enter_context(
                    tc.tile_pool(name="rc_pool", bufs=2))
                ps_kq = ph5.enter_context(
                    tc.tile_pool(name="ps_kq", bufs=3, space="PSUM"))
                ps_attn = ph5.enter_context(
                    tc.tile_pool(name="ps_attn", bufs=3, space="PSUM"))
                ps_ctx = ph5.enter_context(
                    tc.tile_pool(name="ps_ctx", bufs=2, space="PSUM"))

                pts = {}

                def proj_piece(ft):
                    wk = wstr.tile([128, KC, 128], BF16, name="wk", tag="wk")
                    nc.sync.dma_start(
                        out=wk, in_=wqkvT_r[:, :, D + ft * 128:D + ft * 128 + 128])
                    wq = wstr.tile([128, KC, 128], BF16, name="wq", tag="wq")
                    nc.sync.dma_start(
                        out=wq, in_=wqkvT_r[:, :, ft * 128:ft * 128 + 128])
                    for nh in range(2):
                        ps = ps_kq.tile([128, 512], F32, name="pkq",
                                        tag="pkq")
                        for kc in range(KC):
                            nc.tensor.matmul(
                                ps, wk[:, kc, :],
                                xbf[:, kc, nh * 512:nh * 512 + 512],
                                start=(kc == 0), stop=(kc == KC - 1))
                        nc.vector.tensor_scalar(
                            out=kT[:, ft, nh * 512:nh * 512 + 512],
                            in0=ps, scalar1=bk_c[:, ft:ft + 1],
                            scalar2=None, op0=OP.add)
                    ps = ps_kq.tile([128, 512], F32, name="pkq", tag="pkq")
                    for kc in range(KC):
                        nc.tensor.matmul(ps, wq[:, kc, :], xbf[:, kc, 0:SQ],
                                         start=(kc == 0), stop=(kc == KC - 1))
                    nc.vector.tensor_scalar(
                        out=qT[:, ft, :], in0=ps,
                        scalar1=bq_c[:, ft:ft + 1], scalar2=None, op0=OP.add)

                def scores(hh):
                    po = 64 * (hh % 2)
                    ft = hh // 2
                    tiles = []
                    for t in range(NTOK):
                        ps = ps_attn.tile([128, 512], F32, name="ps_s",
                                          tag="ps_s")
                        nc.tensor.matmul(
                            ps, kT[po:po + 64, ft, t * 128:t * 128 + 128],
                            qT[po:po + 64, ft, :], start=True, stop=True)
                        pt = pt_pool.tile([128, 512], BF16, name="pT",
                                          tag="pT")
                        nc.scalar.activation(out=pt, in_=ps, func=AF.Exp,
                                             scale=INV_SQRT)
                        tiles.append(pt)
                    pts[hh] = tiles

                def ctx(hh):
                    po = 64 * (hh % 2)
                    ft = hh // 2
                    pctx = ps_ctx.tile([128, 512], F32, name="ps_c",
                                       tag="ps_c")
                    for t in range(NTOK):
                        nc.tensor.matmul(
                            pctx[0:65, :], Vaug_h[:, t, hh, 0:DH + 1],
                            pts[hh][t], start=(t == 0), stop=(t == NTOK - 1))
                    rsr = rc_pool.tile([1, 512], F32, name="rsr", tag="rsr")
                    nc.vector.tensor_copy(out=rsr, in_=pctx[64:65, :])
                    rch = rc_pool.tile([1, 512], F32, name="rch", tag="rch")
                    nc.vector.reciprocal_approx_fast(out=rch, in_=rsr)
                    rb = rc_pool.tile([64, 512], F32, name="rb", tag="rb")
                    nc.gpsimd.partition_broadcast(rb, rch)
                    nc.vector.tensor_mul(out=ctxT[po:po + 64, ft, :],
                                         in0=pctx[0:64, :], in1=rb)
                    del pts[hh]

                with nc.named_scope("p5_kq_attn"):
                    nc.vector.memset(lo, 0.0)
                    nc.vector.memset(hi, 16.0)
                    proj_piece(0)
                    bisect_iter()
                    proj_piece(1)
                    bisect_iter()
                    bisect_iter()
                    for hh in range(H):
                        if hh % 2 == 0 and hh // 2 + 2 < KC:
                            proj_piece(hh // 2 + 2)
                        scores(hh)
                        if hh >= 1:
                            ctx(hh - 1)
                        bisect_iter()
                    ctx(H - 1)

                # final masking + renorm scale for the sparse path
                with nc.named_scope("p3_mask"):
                    for qt in range(NQT):
                        nc.vector.scalar_tensor_tensor(
                            out=psp[:, qt, :], in0=psp[:, qt, :],
                            scalar=lo[:, qt:qt + 1],
                            in1=psp[:, qt, :], op0=OP.is_ge, op1=OP.mult,
                            accum_out=rs_sp[:, qt:qt + 1])
                    nc.vector.tensor_scalar(out=rs_sp, in0=rs_sp,
                                            scalar1=1e-9, scalar2=None,
                                            op0=OP.add)
                    nc.vector.reciprocal(rcp_sp, rs_sp)
                    nc.vector.tensor_scalar_mul(rcp_sp, rcp_sp, oms_bc)

            kq_stack.close()    # free kT, qT
            scr_stack.close()
            x_stack.close()     # free xTt (f32)
            xbf_stack.close()   # free xbf

            # ========= phase 6: outproj + spmm + fuse + LN1 (qt-outer) =====
            # late broadcast rows: b12, g1, g2, be2 (right-side stack)
            late_bc = est.enter_context(
                tc.tile_pool(name="late_bc", bufs=1, side="right"))
            with contextlib.ExitStack() as brow_stack:
                brow_pool = brow_stack.enter_context(
                    tc.tile_pool(name="brow_pool2", bufs=1, side="right"))
                brow_t = brow_pool.tile([1, 4 * D], F32, name="brow_t2")
                nc.gpsimd.dma_start(out=brow_t,
                                    in_=brow.ap()[:, 3 * D:7 * D])
                b12_bc = late_bc.tile([128, D], F32, name="b12_bc")
                nc.gpsimd.partition_broadcast(b12_bc, brow_t[:, 0:D])
                g1_bc = late_bc.tile([128, D], F32, name="g1_bc")
                nc.gpsimd.partition_broadcast(g1_bc, brow_t[:, D:2 * D])
                g2_bc = late_bc.tile([128, D], F32, name="g2_bc")
                nc.gpsimd.partition_broadcast(g2_bc, brow_t[:, 2 * D:3 * D])
                be2_bc = late_bc.tile([128, D], F32, name="be2_bc")
                nc.gpsimd.partition_broadcast(be2_bc, brow_t[:, 3 * D:4 * D])

            fse = est.enter_context(tc.tile_pool(name="fse", bufs=1,
                                                 side="right"))
            x1 = fse.tile([128, NQT, D], F32, name="x1")
            mv2 = fse.tile([128, NQT, 2], F32, name="mv2")
            stats = fse.tile([128, NQT, 2, 6], F32, name="stats")
            sd = fse.tile([128, NQT], F32, name="sd")
            rstd = fse.tile([128, NQT], F32, name="rstd")

            xln_stack = contextlib.ExitStack()
            xlnT_pool = xln_stack.enter_context(
                tc.tile_pool(name="xlnT_pool", bufs=1, side="right"))
            xlnT = xlnT_pool.tile([128, KC, SQ], BF16, name="xlnT")
            xbf1_stack = contextlib.ExitStack()
            xbf1_pool = xbf1_stack.enter_context(
                tc.tile_pool(name="xbf1_pool", bufs=1, side="right"))
            xbf1 = xbf1_pool.tile([128, NQT, D], BF16, name="xbf1")

            def ln_stats(src_ap, qt):
                for half in range(2):
                    nc.vector.bn_stats(
                        out=stats[:, qt, half, :],
                        in_=src_ap[:, half * 512:half * 512 + 512])
                nc.vector.bn_aggr(out=mv2[:, qt, :], in_=stats[:, qt])
                nc.scalar.activation(out=sd[:, qt:qt + 1],
                                     in_=mv2[:, qt, 1:2], func=AF.Sqrt,
                                     bias=eps_t, scale=1.0)
                nc.vector.reciprocal(rstd[:, qt:qt + 1], sd[:, qt:qt + 1])

            with contextlib.ExitStack() as ph6:
                pm_pool = ph6.enter_context(tc.tile_pool(name="pm_pool",
                                                         bufs=2))
                ps_o = ph6.enter_context(
                    tc.tile_pool(name="ps_o", bufs=4, space="PSUM"))
                ps_sp = ph6.enter_context(
                    tc.tile_pool(name="ps_sp", bufs=2, space="PSUM"))
                ps_tr = ph6.enter_context(
                    tc.tile_pool(name="ps_tr", bufs=2, space="PSUM"))
                with nc.named_scope("p6_fuse"):
                    for qt in range(NQT):
                        qc = slice(qt * 128, qt * 128 + 128)
                        # out_proj (2 halves, ctxT-stationary)
                        po0 = ps_o.tile([128, 512], F32, name="po", tag="po")
                        po1 = ps_o.tile([128, 512], F32, name="po", tag="po")
                        for kc in range(KC):
                            st, sp = (kc == 0), (kc == KC - 1)
                            nc.tensor.matmul(po0, ctxT[:, kc, qc],
                                             woT_s[:, kc, 0:512],
                                             start=st, stop=sp)
                            nc.tensor.matmul(po1, ctxT[:, kc, qc],
                                             woT_s[:, kc, 512:1024],
                                             start=st, stop=sp)
                        # masked-p transposes for this qt
                        pmt = pm_pool.tile([128, NTOK, 128], BF16, name="pmt",
                                           tag="pmt")
                        for t in range(NTOK):
                            pst = ps_tr.tile([128, 128], BF16, name="pst",
                                             tag="pst")
                            nc.tensor.transpose(
                                pst, psp[:, qt, t * 128:t * 128 + 128],
                                ident_b)
                            nc.vector.tensor_copy(out=pmt[:, t, :], in_=pst)
                        # spmm (2 halves)
                        sp0 = ps_sp.tile([128, 512], F32, name="psp2",
                                         tag="psp2")
                        sp1 = ps_sp.tile([128, 512], F32, name="psp2",
                                         tag="psp2")
                        for t in range(NTOK):
                            st, spl = (t == 0), (t == NTOK - 1)
                            nc.tensor.matmul(sp0, pmt[:, t, :],
                                             Vsp[:, t, 0:512],
                                             start=st, stop=spl)
                            nc.tensor.matmul(sp1, pmt[:, t, :],
                                             Vsp[:, t, 512:1024],
                                             start=st, stop=spl)
                        # fuse on DVE: x1 = sig*dense + rcp*spmm + xot
                        xq = x1[:, qt, :]
                        nc.vector.tensor_scalar(
                            out=xq[:, 0:512], in0=po0, scalar1=sig_bc,
                            scalar2=None, op0=OP.mult)
                        nc.vector.tensor_scalar(
                            out=xq[:, 512:1024], in0=po1, scalar1=sig_bc,
                            scalar2=None, op0=OP.mult)
                        nc.vector.tensor_add(xq, xq, xot[:, qt, :])
                        nc.vector.scalar_tensor_tensor(
                            out=xq[:, 0:512], in0=sp0,
                            scalar=rcp_sp[:, qt:qt + 1],
                            in1=xq[:, 0:512], op0=OP.mult, op1=OP.add)
                        nc.vector.scalar_tensor_tensor(
                            out=xq[:, 512:1024], in0=sp1,
                            scalar=rcp_sp[:, qt:qt + 1],
                            in1=xq[:, 512:1024], op0=OP.mult, op1=OP.add)
                        # LN1 (keep x1 raw f32 for the ff2 residual)
                        ln_stats(xq, qt)
                        nc.vector.tensor_scalar(
                            out=xbf1[:, qt, :], in0=xq,
                            scalar1=mv2[:, qt, 0:1],
                            scalar2=rstd[:, qt:qt + 1],
                            op0=OP.subtract, op1=OP.mult)
                        # transpose normalized qt block for ff1
                        for fc in range(KC):
                            pst = ps_tr.tile([128, 128], BF16, name="pst",
                                             tag="pst")
                            nc.tensor.transpose(
                                pst, xbf1[:, qt, fc * 128:fc * 128 + 128],
                                ident_b)
                            nc.vector.tensor_copy(out=xlnT[:, fc, qc],
                                                  in_=pst)

            xbf1_stack.close()
            wo_stack.close()
            xot_stack.close()
            av_stack.close()   # free Vaug, Vsp, ctxT
            sp_stack.close()   # free psp, kspT, qspT

            # f2T resident for qt-outer ff2 (DMA hides under ff1)
            f2_stack = contextlib.ExitStack()
            f2_pool = f2_stack.enter_context(
                tc.tile_pool(name="f2_pool", bufs=1))
            f2_s = f2_pool.tile([128, FC, D], BF16, name="f2_s")
            nc.sync.dma_start(out=f2_s, in_=f2T_r)

            # xg = xhat*g1 + (be1+b2), computed on DVE during ff1
            xg = fse.tile([128, NQT, D], F32, name="xg")

            # ============ ff1 + relu ============
            h1_stack = contextlib.ExitStack()
            h1_pool = h1_stack.enter_context(
                tc.tile_pool(name="h1_pool", bufs=1))
            h1T = h1_pool.tile([128, FC, SQ], BF16, name="h1T")
            with contextlib.ExitStack() as ph9:
                w3str = ph9.enter_context(tc.tile_pool(name="w3str", bufs=2))
                ps_f1 = ph9.enter_context(
                    tc.tile_pool(name="ps_f1", bufs=4, space="PSUM"))
                with nc.named_scope("p9_ff1"):
                    for jj in range(16):
                        wt = w3str.tile([128, KC, 256], BF16, name="w1t",
                                        tag="w3")
                        f0 = jj * 256
                        eng = nc.scalar if jj % 2 == 0 else nc.sync
                        eng.dma_start(out=wt, in_=f1T_r[:, :, f0:f0 + 256])
                        for fi in range(2):
                            dft = jj * 2 + fi
                            ps = ps_f1.tile([128, 512], F32, name="ps_f",
                                            tag="ps_f")
                            for kc in range(KC):
                                nc.tensor.matmul(
                                    ps, wt[:, kc, fi * 128:fi * 128 + 128],
                                    xlnT[:, kc, :],
                                    start=(kc == 0), stop=(kc == KC - 1))
                            nc.scalar.activation(
                                out=h1T[:, dft, :], in_=ps, func=AF.Relu,
                                bias=b1_c[:, dft:dft + 1], scale=1.0)
                        if jj < 2 * NQT and jj % 2 == 1:
                            # xg for qt = jj//2, hidden under ff1
                            qt = jj // 2
                            nc.vector.tensor_scalar(
                                out=xg[:, qt, :], in0=x1[:, qt, :],
                                scalar1=mv2[:, qt, 0:1],
                                scalar2=rstd[:, qt:qt + 1],
                                op0=OP.subtract, op1=OP.mult)
                            nc.vector.tensor_mul(xg[:, qt, :], xg[:, qt, :],
                                                 g1_bc)
                            nc.vector.tensor_add(xg[:, qt, :], xg[:, qt, :],
                                                 b12_bc)
            xln_stack.close()

            # ============ ff2 (qt-outer) + residual + LN2 + out ============
            with contextlib.ExitStack() as ph10:
                ps_f2 = ph10.enter_context(
                    tc.tile_pool(name="ps_f2", bufs=4, space="PSUM"))
                ot_pool = ph10.enter_context(
                    tc.tile_pool(name="ot_pool", bufs=2))
                with nc.named_scope("p10_ff2"):
                    for qt in range(NQT):
                        qc = slice(qt * 128, qt * 128 + 128)
                        pg0 = ps_f2.tile([128, 512], F32, name="pg", tag="pg")
                        pg1 = ps_f2.tile([128, 512], F32, name="pg", tag="pg")
                        for kc in range(FC):
                            st, sp = (kc == 0), (kc == FC - 1)
                            nc.tensor.matmul(pg0, h1T[:, kc, qc],
                                             f2_s[:, kc, 0:512],
                                             start=st, stop=sp)
                            nc.tensor.matmul(pg1, h1T[:, kc, qc],
                                             f2_s[:, kc, 512:1024],
                                             start=st, stop=sp)
                        x2 = x1[:, qt, :]
                        nc.vector.tensor_add(x2[:, 0:512], pg0,
                                             xg[:, qt, 0:512])
                        nc.vector.tensor_add(x2[:, 512:1024], pg1,
                                             xg[:, qt, 512:1024])
                        ln_stats(x2, qt)
                        ot = ot_pool.tile([128, D], F32, name="out_t",
                                          tag="out_t")
                        nc.vector.tensor_scalar(
                            out=ot, in0=x2, scalar1=mv2[:, qt, 0:1],
                            scalar2=rstd[:, qt:qt + 1],
                            op0=OP.subtract, op1=OP.mult)
                        nc.vector.tensor_mul(ot, ot, g2_bc)
                        nc.vector.tensor_add(ot, ot, be2_bc)
                        nc.scalar.dma_start(
                            out=out.ap()[qt * 128:qt * 128 + 128, :], in_=ot)
            h1_stack.close()
            f2_stack.close()

    nc.compile()
    return nc


def _prep_inputs(src, in_proj_w, in_proj_b, out_proj_w, out_proj_b,
                 Qp_w, Qp_b, Kp_w, Kp_b, Vp_w, Vp_b, lam,
                 ff1_w, ff1_b, ff2_w, ff2_b, ln1_g, ln1_b, ln2_g, ln2_b):
    import ml_dtypes
    f = np.float32
    A = lambda x: np.ascontiguousarray(x, dtype=f)
    AB = lambda x: np.ascontiguousarray(np.asarray(x, dtype=f),
                                        dtype=ml_dtypes.bfloat16)
    in_proj_w = np.asarray(in_proj_w, dtype=f)
    ff1_w = np.asarray(ff1_w, dtype=f)
    ln1_g = np.asarray(ln1_g, dtype=f)
    ln1_b = np.asarray(ln1_b, dtype=f)
    # fold ln1 gamma into ff1 weights, ln1 beta into ff1 bias
    f1_eff = ff1_w * ln1_g[None, :]            # [DFF, D]
    b1_eff = np.asarray(ff1_b, dtype=f) + ff1_w @ ln1_b

    # packed per-partition bias columns [128, 50]
    bcols = np.zeros((128, 50), dtype=f)
    qk_bias = np.asarray(in_proj_b, dtype=f)
    bcols[:, 0:8] = qk_bias[0:D].reshape(8, 128).T
    bcols[:, 8:16] = qk_bias[D:2 * D].reshape(8, 128).T
    bcols[:, 16:48] = b1_eff.reshape(32, 128).T
    bcols[0:64, 48] = np.asarray(Qp_b, dtype=f)
    bcols[0:64, 49] = np.asarray(Kp_b, dtype=f)

    # packed broadcast rows [1, 7D]: bv, bvp, bo, b12, g1, g2, be2
    brow = np.concatenate([
        qk_bias[2 * D:3 * D],
        np.asarray(Vp_b, dtype=f),
        np.asarray(out_proj_b, dtype=f),
        ln1_b + np.asarray(ff2_b, dtype=f),
        ln1_g,
        np.asarray(ln2_g, dtype=f),
        np.asarray(ln2_b, dtype=f),
    ]).reshape(1, 7 * D)

    # qkp packed partition-major: [128, kc, 2R] contiguous
    qkpT = np.concatenate([np.asarray(Qp_w).T, np.asarray(Kp_w).T], axis=1)
    qkp_pm = np.ascontiguousarray(
        qkpT.reshape(KC, 128, 2 * R).transpose(1, 0, 2).reshape(128, -1),
        dtype=f)

    shared = {
        "wqkvT": AB(in_proj_w.T),
        "woT": AB(np.asarray(out_proj_w).T),
        "vpT": AB(np.asarray(Vp_w).T),
        "qkp_pm": qkp_pm,
        "f1T": AB(f1_eff.T),
        "f2T": AB(np.asarray(ff2_w).T),
        "bcols": bcols,
        "brow": A(brow),
        "lam": A(np.asarray(lam)).reshape(1, 1),
    }
    in_maps = []
    for core in range(8):
        b, h = core // 2, core % 2
        srcb = np.asarray(src[b])
        xTb = srcb.T
        if h == 1:
            # own-query columns first (key order is irrelevant to attention)
            xTb = np.concatenate([xTb[:, SQ:], xTb[:, :SQ]], axis=1)
        m = dict(shared)
        m["xT"] = A(xTb)
        m["x_own"] = A(srcb[h * SQ:(h + 1) * SQ, :])
        in_maps.append(m)
    return in_maps


def _run(inputs, trace=False):
    if "nc" not in _cached:
        _cached["nc"] = _build()
    nc = _cached["nc"]
    in_maps = _prep_inputs(**inputs)
    res = run_bass_kernel_spmd(nc, in_maps, core_ids=list(range(8)),
                               trace=trace)
    out = np.empty((B, S, D), np.float32)
    for core in range(8):
        b, h = core // 2, core % 2
        out[b, h * SQ:(h + 1) * SQ, :] = res.results[core]["out"]
    return out, res


def kernel(**inputs) -> np.ndarray:
    out, _ = _run(inputs, trace=False)
    return out


# revision 22
# speedup vs baseline: 1.1183x; 1.0232x over previous
"""Trainium2 Bass kernel for the EnhancedEncoderLayer (dense MHA + low-rank
top-k sparse attention + FFN, two layernorms).

Sharding: 8 cores = (batch b in 0..3) x (query-half h in {0,1}). Each core
computes output rows [b, h*512:(h+1)*512, :]. K/V-side projections are
computed redundantly per batch pair (no cross-core communication).

The host permutes src[b].T columns so each core's own query tokens are
columns 0..511 (attention contracts over all keys, so key order is
irrelevant); this keeps the SPMD program identical across cores.

v2 design notes:
- v/vsp projections are x-stationary (lhsT = xT chunk), producing
  token-major Vaug/Vsp directly -- no PE transposes, no ACT copies.
- dense attention is software-pipelined: scores(h) / proj filler /
  ctx(h-1), so ACT exp latency never stalls the in-order PE queue.
- the sparse path runs in bf16: exp writes bf16 psp, the top-k threshold
  bisection scans at 2x DVE rate with 18 iterations, spmm is bf16.
- outproj+spmm+fuse+LN1 run qt-outer so LN1 overlaps matmuls; ff2 is
  qt-outer against an SBUF-resident f2T with LN2+output pipelined per qt.
- ln1 gamma/beta are folded into ff1 weights/bias host-side.
- all host tensors are laid out so every DMA is contiguous per partition.
"""
import sys
import os
import contextlib

for _p in ('/opt/trn_rl_repo',):
    if _p not in sys.path:
        sys.path.insert(0, _p)

import numpy as np
import concourse.bacc as bacc
import concourse.tile as tile
from concourse import mybir
from concourse.bass_utils import run_bass_kernel_spmd
from concourse.masks import make_identity

F32 = mybir.dt.float32
F32R = mybir.dt.float32r
BF16 = mybir.dt.bfloat16
AF = mybir.ActivationFunctionType
OP = mybir.AluOpType

B, S, D, H, R, DFF = 4, 1024, 1024, 16, 64, 4096
DH = D // H          # 64
SQ = S // 2          # 512 own queries per core
KK = max(1, int(S * 0.2))   # 204
KC = D // 128        # 8 contraction chunks over D
FC = DFF // 128      # 32 chunks over DFF
NQT = SQ // 128      # 4 query tiles
NTOK = S // 128      # 8 token tiles
BISECT_ITERS = 19
INV_SQRT = 0.125     # 1/sqrt(DH) == 1/sqrt(R)

_cached = {}


def _build():
    nc = bacc.Bacc()

    def din(name, shape, dt=F32):
        return nc.declare_dram_parameter(name, list(shape), dt, isOutput=False)

    xT = din("xT", [D, S])            # src[b].T, own-query columns first
    x_own = din("x_own", [SQ, D])     # own rows, token-major
    wqkvT = din("wqkvT", [D, 3 * D], BF16)
    woT = din("woT", [D, D], BF16)
    vpT = din("vpT", [D, D], BF16)
    qkp_pm = din("qkp_pm", [128, KC * 2 * R])   # partition-major packed
    f1T = din("f1T", [D, DFF], BF16)  # pre-scaled by ln1 gamma
    f2T = din("f2T", [DFF, D], BF16)
    # packed per-partition bias columns: [0:8]=q, [8:16]=k, [16:48]=b1_eff,
    # [48]=bqp (rows 0:64), [49]=bkp (rows 0:64)
    bcols = din("bcols", [128, 50])
    # packed broadcast rows: [bv, bvp, bo, b12, g1, g2, be2]
    brow = din("brow", [1, 7 * D])
    lam = din("lam", [1, 1])
    out = nc.declare_dram_parameter("out", [SQ, D], F32, isOutput=True)

    xT_r = xT.ap().bitcast(F32R).rearrange("(kc p) s -> p kc s", p=128)
    wqkvT_r = wqkvT.ap().rearrange("(kc p) f -> p kc f", p=128)
    woT_r = woT.ap().rearrange("(kc p) f -> p kc f", p=128)
    vpT_r = vpT.ap().rearrange("(kc p) f -> p kc f", p=128)
    qkp_r = qkp_pm.ap().bitcast(F32R).rearrange("p (kc f) -> p kc f", f=2 * R)
    f1T_r = f1T.ap().rearrange("(kc p) f -> p kc f", p=128)
    f2T_r = f2T.ap().rearrange("(kc p) f -> p kc f", p=128)

    with tile.TileContext(nc) as tc:
        est = contextlib.ExitStack()
        with est:
            # ---------------- constants ----------------
            consts = est.enter_context(tc.tile_pool(name="consts", bufs=1))

            ident_f = consts.tile([128, 128], F32, name="ident_f")
            make_identity(nc, ident_f)
            ident_b = consts.tile([128, 128], BF16, name="ident_b")
            nc.vector.tensor_copy(out=ident_b, in_=ident_f)

            eps_t = consts.tile([128, 1], F32, name="eps_t")
            nc.vector.memset(eps_t, 1e-5)
            ones1 = consts.tile([128, 1], F32, name="ones1")
            nc.vector.memset(ones1, 1.0)
            ones1b = consts.tile([128, 1], BF16, name="ones1b")
            nc.vector.memset(ones1b, 1.0)
            ones16b = consts.tile([128, 16], BF16, name="ones16b")
            nc.vector.memset(ones16b, 1.0)

            lam_t = consts.tile([1, 1], F32, name="lam_t")
            nc.sync.dma_start(out=lam_t, in_=lam.ap())
            sg_t = consts.tile([1, 1], F32, name="sg_t")
            nc.scalar.activation(out=sg_t, in_=lam_t, func=AF.Sigmoid)
            sig_bc = consts.tile([128, 1], F32, name="sig_bc")
            nc.gpsimd.partition_broadcast(sig_bc, sg_t)
            oms_bc = consts.tile([128, 1], F32, name="oms_bc")
            nc.vector.tensor_sub(oms_bc, ones1, sig_bc)

            # packed bias columns (one contiguous DMA)
            bcols_t = consts.tile([128, 50], F32, name="bcols_t")
            nc.gpsimd.dma_start(out=bcols_t, in_=bcols.ap())
            bq_c = bcols_t[:, 0:8]
            bk_c = bcols_t[:, 8:16]
            b1_c = bcols_t[:, 16:48]
            bqp_c = bcols_t[0:64, 48:49]
            bkp_c = bcols_t[0:64, 49:50]

            # bisect state
            bis = est.enter_context(tc.tile_pool(name="bis", bufs=1))
            lo = bis.tile([128, NQT], F32, name="lo")
            hi = bis.tile([128, NQT], F32, name="hi")
            mid = bis.tile([128, NQT], F32, name="mid")
            cnts = bis.tile([128, NQT], F32, name="cnts")
            pred = bis.tile([128, NQT], mybir.dt.uint32, name="pred")
            rs_sp = bis.tile([128, NQT], F32, name="rs_sp")
            rcp_sp = bis.tile([128, NQT], F32, name="rcp_sp")

            # long-lived activations
            sp_stack = contextlib.ExitStack()
            sp_pool = sp_stack.enter_context(
                tc.tile_pool(name="sp_pool", bufs=1))
            psp = sp_pool.tile([128, NQT, S], BF16, name="psp")
            kspT = sp_pool.tile([64, S], F32R, name="kspT")
            qspT = sp_pool.tile([64, SQ], F32R, name="qspT")

            av_stack = contextlib.ExitStack()
            av_pool = av_stack.enter_context(
                tc.tile_pool(name="av_pool", bufs=1))
            Vaug = av_pool.tile([128, NTOK, H * (DH + 1)], BF16, name="Vaug")
            Vsp = av_pool.tile([128, NTOK, D], BF16, name="Vsp")
            ctxT = av_pool.tile([128, KC, SQ], BF16, name="ctxT")

            Vaug_h = Vaug.rearrange("p t (h c) -> p t h c", c=DH + 1)
            for t in range(NTOK):
                nc.vector.tensor_copy(out=Vaug_h[:, t, :, DH:DH + 1],
                                      in_=ones16b)

            xot_stack = contextlib.ExitStack()
            xot_pool = xot_stack.enter_context(tc.tile_pool(name="xot_pool",
                                                            bufs=1))
            xot = xot_pool.tile([128, NQT, D], F32, name="xot")

            # out_proj weights (DMA issued later, after the startup crunch)
            wo_stack = contextlib.ExitStack()
            wo_pool = wo_stack.enter_context(
                tc.tile_pool(name="wo_pool", bufs=1))
            woT_s = wo_pool.tile([128, KC, D], BF16, name="woT_s")

            # =========== phase 0/1: input loads, sparse + v projections ====
            xbf_stack = contextlib.ExitStack()
            xbf_pool = xbf_stack.enter_context(
                tc.tile_pool(name="xbf_pool", bufs=1))
            xbf = xbf_pool.tile([128, KC, S], BF16, name="xbf")
            x_stack = contextlib.ExitStack()
            xt_pool = x_stack.enter_context(tc.tile_pool(name="xt_pool",
                                                         bufs=1))
            xTt = xt_pool.tile([128, KC, S], F32R, name="xTt")

            # early broadcast rows: bv, bvp, sig*bo
            early_stack = contextlib.ExitStack()
            early_bc = early_stack.enter_context(
                tc.tile_pool(name="early_bc", bufs=1))
            with contextlib.ExitStack() as brow_stack:
                brow_pool = brow_stack.enter_context(
                    tc.tile_pool(name="brow_pool", bufs=1))
                brow_t = brow_pool.tile([1, 3 * D], F32, name="brow_t")
                nc.gpsimd.dma_start(out=brow_t, in_=brow.ap()[:, 0:3 * D])
                bv_bc = early_bc.tile([128, D], F32, name="bv_bc")
                nc.gpsimd.partition_broadcast(bv_bc, brow_t[:, 0 * D:1 * D])
                bvp_bc = early_bc.tile([128, D], F32, name="bvp_bc")
                nc.gpsimd.partition_broadcast(bvp_bc, brow_t[:, 1 * D:2 * D])
                bo_sig = early_bc.tile([128, D], F32, name="bo_sig")
                nc.gpsimd.partition_broadcast(bo_sig, brow_t[:, 2 * D:3 * D])
                nc.vector.tensor_scalar_mul(bo_sig, bo_sig, sig_bc)

            with contextlib.ExitStack() as ph0:
                wsp_pool = ph0.enter_context(
                    tc.tile_pool(name="wsp_pool", bufs=1))
                ps_proj = ph0.enter_context(
                    tc.tile_pool(name="ps_proj", bufs=3, space="PSUM"))

                qkpt = wsp_pool.tile([128, KC, 2 * R], F32R, name="qkpt")
                nc.sync.dma_start(out=qkpt, in_=qkp_r)
                qpt = qkpt[:, :, 0:R]
                kpt = qkpt[:, :, R:2 * R]
                for kc in range(KC):
                    eng = nc.scalar if kc % 2 == 0 else nc.sync
                    eng.dma_start(out=xTt[:, kc, :], in_=xT_r[:, kc, :])

                # ---- sparse projections + scores (f32r) ----
                with nc.named_scope("p0_ksp_qsp"):
                    for nh in range(2):
                        ps = ps_proj.tile([128, 512], F32, name="ps",
                                          tag="ps")
                        for kc in range(KC):
                            nc.tensor.matmul(
                                ps[0:64, :], kpt[:, kc, :],
                                xTt[:, kc, nh * 512:nh * 512 + 512],
                                start=(kc == 0), stop=(kc == KC - 1))
                        nc.scalar.activation(
                            out=kspT[:, nh * 512:nh * 512 + 512],
                            in_=ps[0:64, :], func=AF.Identity, bias=bkp_c,
                            scale=1.0)
                    ps = ps_proj.tile([128, 512], F32, name="ps", tag="ps")
                    for kc in range(KC):
                        nc.tensor.matmul(ps[0:64, :], qpt[:, kc, :],
                                         xTt[:, kc, 0:SQ],
                                         start=(kc == 0), stop=(kc == KC - 1))
                    nc.scalar.activation(out=qspT, in_=ps[0:64, :],
                                         func=AF.Identity, bias=bqp_c,
                                         scale=1.0)

                with nc.named_scope("p2_ssp"):
                    for qt in range(NQT):
                        for nh in range(2):
                            ps = ps_proj.tile([128, 512], F32, name="ps",
                                              tag="ps")
                            nc.tensor.matmul(
                                ps, qspT[:, qt * 128:qt * 128 + 128],
                                kspT[:, nh * 512:nh * 512 + 512],
                                start=True, stop=True)
                            nc.scalar.activation(
                                out=psp[:, qt, nh * 512:nh * 512 + 512],
                                in_=ps, func=AF.Exp, scale=INV_SQRT)

                # cast xT to bf16 for the trunk projections
                with nc.named_scope("p0_cast"):
                    for kc in range(KC):
                        nc.vector.tensor_copy(out=xbf[:, kc, :],
                                              in_=xTt[:, kc, :])

                # own-token residual (+ sig*bo)
                for qt in range(NQT):
                    nc.scalar.dma_start(
                        out=xot[:, qt, :],
                        in_=x_own.ap()[qt * 128:qt * 128 + 128, :])
                    nc.gpsimd.tensor_add(xot[:, qt, :], xot[:, qt, :],
                                         bo_sig)

            # ---- v/vsp x-stationary projections -> token-major ----
            with contextlib.ExitStack() as ph4:
                # reopened weight pool (wv_s, wvp_s still live via av? no --
                # keep them in this scope)
                wv_pool2 = ph4.enter_context(
                    tc.tile_pool(name="wv_pool2", bufs=1))
                wv_s = wv_pool2.tile([128, KC, D], BF16, name="wv_s2")
                nc.sync.dma_start(out=wv_s, in_=wqkvT_r[:, :, 2 * D:3 * D])
                wvp_s = wv_pool2.tile([128, KC, D], BF16, name="wvp_s2")
                nc.sync.dma_start(out=wvp_s, in_=vpT_r)
                ps_v = ph4.enter_context(
                    tc.tile_pool(name="ps_v", bufs=8, space="PSUM"))
                with nc.named_scope("p4_v"):
                    for t in range(NTOK):
                        pva0 = ps_v.tile([128, 512], F32, name="pv", tag="pv")
                        pva1 = ps_v.tile([128, 512], F32, name="pv", tag="pv")
                        pvs0 = ps_v.tile([128, 512], F32, name="pv", tag="pv")
                        pvs1 = ps_v.tile([128, 512], F32, name="pv", tag="pv")
                        for kc in range(KC):
                            xck = xbf[:, kc, t * 128:t * 128 + 128]
                            st = (kc == 0)
                            sp = (kc == KC - 1)
                            nc.tensor.matmul(pva0, xck, wv_s[:, kc, 0:512],
                                             start=st, stop=sp)
                            nc.tensor.matmul(pva1, xck, wv_s[:, kc, 512:1024],
                                             start=st, stop=sp)
                            nc.tensor.matmul(pvs0, xck, wvp_s[:, kc, 0:512],
                                             start=st, stop=sp)
                            nc.tensor.matmul(pvs1, xck, wvp_s[:, kc, 512:1024],
                                             start=st, stop=sp)
                        nc.vector.tensor_add(
                            Vaug_h[:, t, 0:8, 0:DH], pva0, bv_bc[:, 0:512])
                        nc.vector.tensor_add(
                            Vaug_h[:, t, 8:16, 0:DH], pva1, bv_bc[:, 512:1024])
                        nc.vector.tensor_add(
                            Vsp[:, t, 0:512], pvs0, bvp_bc[:, 0:512])
                        nc.vector.tensor_add(
                            Vsp[:, t, 512:1024], pvs1, bvp_bc[:, 512:1024])
            early_stack.close()   # free bv_bc, bvp_bc, bo_sig

            # out_proj weights resident (used in p6)
            nc.sync.dma_start(out=woT_s, in_=woT_r)

            # bisect scratch: lives until after the masking pass
            scr_stack = contextlib.ExitStack()
            scr_pool = scr_stack.enter_context(
                tc.tile_pool(name="scr", bufs=4))

            def bisect_iter():
                # one threshold-bisection step; qt 0-2 scan on DVE, qt 3 on
                # GpSimd (SBUF-only engine, otherwise idle here)
                nc.vector.tensor_add(mid, lo, hi)
                nc.vector.tensor_scalar_mul(mid, mid, 0.5)
                for qt in range(NQT):
                    scr = scr_pool.tile([128, S], BF16, name="scr",
                                        tag="scr")
                    nc.vector.scalar_tensor_tensor(
                        out=scr, in0=psp[:, qt, :],
                        scalar=mid[:, qt:qt + 1],
                        in1=ones1b.to_broadcast([128, S]),
                        op0=OP.is_ge, op1=OP.mult,
                        accum_out=cnts[:, qt:qt + 1])
                nc.vector.tensor_scalar(out=pred, in0=cnts,
                                        scalar1=float(KK),
                                        scalar2=None, op0=OP.is_ge)
                nc.vector.copy_predicated(lo, pred, mid)
                nc.vector.tensor_scalar(out=pred, in0=cnts,
                                        scalar1=float(KK),
                                        scalar2=None, op0=OP.is_lt)
                nc.vector.copy_predicated(hi, pred, mid)

            # ======== phase 5: k/q projections + pipelined attention =======
            kq_stack = contextlib.ExitStack()
            kq_pool = kq_stack.enter_context(
                tc.tile_pool(name="kq_pool", bufs=1))
            kT = kq_pool.tile([128, KC, S], BF16, name="kT")
            qT = kq_pool.tile([128, KC, SQ], BF16, name="qT")
            with contextlib.ExitStack() as ph5:
                wstr = ph5.enter_context(tc.tile_pool(name="wstr", bufs=3))
                pt_pool = ph5.enter_context(
                    tc.tile_pool(name="pt_pool", bufs=16))
                rc_pool = ph5.enter_context(
                    tc.tile_pool(name="rc_pool", bufs=2))
                ps_kq = ph5.enter_context(
                    tc.tile_pool(name="ps_kq", bufs=3, space="PSUM"))
                ps_attn = ph5.enter_context(
                    tc.tile_pool(name="ps_attn", bufs=3, space="PSUM"))
                ps_ctx = ph5.enter_context(
                    tc.tile_pool(name="ps_ctx", bufs=2, space="PSUM"))

                pts = {}

                def proj_piece(ft):
                    wk = wstr.tile([128, KC, 128], BF16, name="wk", tag="wk")
                    nc.sync.dma_start(
                        out=wk, in_=wqkvT_r[:, :, D + ft * 128:D + ft * 128 + 128])
                    wq = wstr.tile([128, KC, 128], BF16, name="wq", tag="wq")
                    nc.sync.dma_start(
                        out=wq, in_=wqkvT_r[:, :, ft * 128:ft * 128 + 128])
                    for nh in range(2):
                        ps = ps_kq.tile([128, 512], F32, name="pkq",
                                        tag="pkq")
                        for kc in range(KC):
                            nc.tensor.matmul(
                                ps, wk[:, kc, :],
                                xbf[:, kc, nh * 512:nh * 512 + 512],
                                start=(kc == 0), stop=(kc == KC - 1))
                        nc.vector.tensor_scalar(
                            out=kT[:, ft, nh * 512:nh * 512 + 512],
                            in0=ps, scalar1=bk_c[:, ft:ft + 1],
                            scalar2=None, op0=OP.add)
                    ps = ps_kq.tile([128, 512], F32, name="pkq", tag="pkq")
                    for kc in range(KC):
                        nc.tensor.matmul(ps, wq[:, kc, :], xbf[:, kc, 0:SQ],
                                         start=(kc == 0), stop=(kc == KC - 1))
                    nc.vector.tensor_scalar(
                        out=qT[:, ft, :], in0=ps,
                        scalar1=bq_c[:, ft:ft + 1], scalar2=None, op0=OP.add)

                def scores(hh):
                    po = 64 * (hh % 2)
                    ft = hh // 2
                    tiles = []
                    for t in range(NTOK):
                        ps = ps_attn.tile([128, 512], F32, name="ps_s",
                                          tag="ps_s")
                        nc.tensor.matmul(
                            ps, kT[po:po + 64, ft, t * 128:t * 128 + 128],
                            qT[po:po + 64, ft, :], start=True, stop=True)
                        pt = pt_pool.tile([128, 512], BF16, name="pT",
                                          tag="pT")
                        nc.scalar.activation(out=pt, in_=ps, func=AF.Exp,
                                             scale=INV_SQRT)
                        tiles.append(pt)
                    pts[hh] = tiles

                def ctx(hh):
                    po = 64 * (hh % 2)
                    ft = hh // 2
                    pctx = ps_ctx.tile([128, 512], F32, name="ps_c",
                                       tag="ps_c")
                    for t in range(NTOK):
                        nc.tensor.matmul(
                            pctx[0:65, :], Vaug_h[:, t, hh, 0:DH + 1],
                            pts[hh][t], start=(t == 0), stop=(t == NTOK - 1))
                    rsr = rc_pool.tile([1, 512], F32, name="rsr", tag="rsr")
                    nc.vector.tensor_copy(out=rsr, in_=pctx[64:65, :])
                    rch = rc_pool.tile([1, 512], F32, name="rch", tag="rch")
                    nc.vector.reciprocal_approx_fast(out=rch, in_=rsr)
                    rb = rc_pool.tile([64, 512], F32, name="rb", tag="rb")
                    nc.gpsimd.partition_broadcast(rb, rch)
                    nc.vector.tensor_mul(out=ctxT[po:po + 64, ft, :],
                                         in0=pctx[0:64, :], in1=rb)
                    del pts[hh]

                with nc.named_scope("p5_kq_attn"):
                    nc.vector.memset(lo, 0.0)
                    nc.vector.memset(hi, 16.0)
                    proj_piece(0)
                    bisect_iter()
                    proj_piece(1)
                    bisect_iter()
                    bisect_iter()
                    for hh in range(H):
                        if hh % 2 == 0 and hh // 2 + 2 < KC:
                            proj_piece(hh // 2 + 2)
                        scores(hh)
                        if hh >= 1:
                            ctx(hh - 1)
                        bisect_iter()
                    ctx(H - 1)

                # final masking + renorm scale for the sparse path
                with nc.named_scope("p3_mask"):
                    for qt in range(NQT):
                        nc.vector.scalar_tensor_tensor(
                            out=psp[:, qt, :], in0=psp[:, qt, :],
                            scalar=lo[:, qt:qt + 1],
                            in1=psp[:, qt, :], op0=OP.is_ge, op1=OP.mult,
                            accum_out=rs_sp[:, qt:qt + 1])
                    nc.vector.tensor_scalar(out=rs_sp, in0=rs_sp,
                                            scalar1=1e-9, scalar2=None,
                                            op0=OP.add)
                    nc.vector.reciprocal(rcp_sp, rs_sp)
                    nc.vector.tensor_scalar_mul(rcp_sp, rcp_sp, oms_bc)

            kq_stack.close()    # free kT, qT
            scr_stack.close()
            x_stack.close()     # free xTt (f32)
            xbf_stack.close()   # free xbf

            # ========= phase 6: outproj + spmm + fuse + LN1 (qt-outer) =====
            # late broadcast rows: b12, g1, g2, be2 (right-side stack)
            late_bc = est.enter_context(
                tc.tile_pool(name="late_bc", bufs=1, side="right"))
            with contextlib.ExitStack() as brow_stack:
                brow_pool = brow_stack.enter_context(
                    tc.tile_pool(name="brow_pool2", bufs=1, side="right"))
                brow_t = brow_pool.tile([1, 4 * D], F32, name="brow_t2")
                nc.gpsimd.dma_start(out=brow_t,
                                    in_=brow.ap()[:, 3 * D:7 * D])
                b12_bc = late_bc.tile([128, D], F32, name="b12_bc")
                nc.gpsimd.partition_broadcast(b12_bc, brow_t[:, 0:D])
                g1_bc = late_bc.tile([128, D], F32, name="g1_bc")
                nc.gpsimd.partition_broadcast(g1_bc, brow_t[:, D:2 * D])
                g2_bc = late_bc.tile([128, D], F32, name="g2_bc")
                nc.gpsimd.partition_broadcast(g2_bc, brow_t[:, 2 * D:3 * D])
                be2_bc = late_bc.tile([128, D], F32, name="be2_bc")
                nc.gpsimd.partition_broadcast(be2_bc, brow_t[:, 3 * D:4 * D])

            fse = est.enter_context(tc.tile_pool(name="fse", bufs=1,
                                                 side="right"))
            x1 = fse.tile([128, NQT, D], F32, name="x1")
            mv2 = fse.tile([128, NQT, 2], F32, name="mv2")
            stats = fse.tile([128, NQT, 2, 6], F32, name="stats")
            sd = fse.tile([128, NQT], F32, name="sd")
            rstd = fse.tile([128, NQT], F32, name="rstd")

            xln_stack = contextlib.ExitStack()
            xlnT_pool = xln_stack.enter_context(
                tc.tile_pool(name="xlnT_pool", bufs=1, side="right"))
            xlnT = xlnT_pool.tile([128, KC, SQ], BF16, name="xlnT")
            xbf1_stack = contextlib.ExitStack()
            xbf1_pool = xbf1_stack.enter_context(
                tc.tile_pool(name="xbf1_pool", bufs=1, side="right"))
            xbf1 = xbf1_pool.tile([128, NQT, D], BF16, name="xbf1")

            def ln_stats(src_ap, qt):
                for half in range(2):
                    nc.vector.bn_stats(
                        out=stats[:, qt, half, :],
                        in_=src_ap[:, half * 512:half * 512 + 512])
                nc.vector.bn_aggr(out=mv2[:, qt, :], in_=stats[:, qt])
                nc.scalar.activation(out=sd[:, qt:qt + 1],
                                     in_=mv2[:, qt, 1:2], func=AF.Sqrt,
                                     bias=eps_t, scale=1.0)
                nc.vector.reciprocal(rstd[:, qt:qt + 1], sd[:, qt:qt + 1])

            with contextlib.ExitStack() as ph6:
                pm_pool = ph6.enter_context(tc.tile_pool(name="pm_pool",
                                                         bufs=2))
                ps_o = ph6.enter_context(
                    tc.tile_pool(name="ps_o", bufs=4, space="PSUM"))
                ps_sp = ph6.enter_context(
                    tc.tile_pool(name="ps_sp", bufs=2, space="PSUM"))
                ps_tr = ph6.enter_context(
                    tc.tile_pool(name="ps_tr", bufs=2, space="PSUM"))
                with nc.named_scope("p6_fuse"):
                    for qt in range(NQT):
                        qc = slice(qt * 128, qt * 128 + 128)
                        # out_proj (2 halves, ctxT-stationary)
                        po0 = ps_o.tile([128, 512], F32, name="po", tag="po")
                        po1 = ps_o.tile([128, 512], F32, name="po", tag="po")
                        for kc in range(KC):
                            st, sp = (kc == 0), (kc == KC - 1)
                            nc.tensor.matmul(po0, ctxT[:, kc, qc],
                                             woT_s[:, kc, 0:512],
                                             start=st, stop=sp)
                            nc.tensor.matmul(po1, ctxT[:, kc, qc],
                                             woT_s[:, kc, 512:1024],
                                             start=st, stop=sp)
                        # masked-p transposes for this qt
                        pmt = pm_pool.tile([128, NTOK, 128], BF16, name="pmt",
                                           tag="pmt")
                        for t in range(NTOK):
                            pst = ps_tr.tile([128, 128], BF16, name="pst",
                                             tag="pst")
                            nc.tensor.transpose(
                                pst, psp[:, qt, t * 128:t * 128 + 128],
                                ident_b)
                            nc.vector.tensor_copy(out=pmt[:, t, :], in_=pst)
                        # spmm (2 halves)
                        sp0 = ps_sp.tile([128, 512], F32, name="psp2",
                                         tag="psp2")
                        sp1 = ps_sp.tile([128, 512], F32, name="psp2",
                                         tag="psp2")
                        for t in range(NTOK):
                            st, spl = (t == 0), (t == NTOK - 1)
                            nc.tensor.matmul(sp0, pmt[:, t, :],
                                             Vsp[:, t, 0:512],
                                             start=st, stop=spl)
                            nc.tensor.matmul(sp1, pmt[:, t, :],
                                             Vsp[:, t, 512:1024],
                                             start=st, stop=spl)
                        # fuse on DVE: x1 = sig*dense + rcp*spmm + xot
                        xq = x1[:, qt, :]
                        nc.vector.tensor_scalar(
                            out=xq[:, 0:512], in0=po0, scalar1=sig_bc,
                            scalar2=None, op0=OP.mult)
                        nc.vector.tensor_scalar(
                            out=xq[:, 512:1024], in0=po1, scalar1=sig_bc,
                            scalar2=None, op0=OP.mult)
                        nc.vector.tensor_add(xq, xq, xot[:, qt, :])
                        nc.vector.scalar_tensor_tensor(
                            out=xq[:, 0:512], in0=sp0,
                            scalar=rcp_sp[:, qt:qt + 1],
                            in1=xq[:, 0:512], op0=OP.mult, op1=OP.add)
                        nc.vector.scalar_tensor_tensor(
                            out=xq[:, 512:1024], in0=sp1,
                            scalar=rcp_sp[:, qt:qt + 1],
                            in1=xq[:, 512:1024], op0=OP.mult, op1=OP.add)
                        # LN1 (keep x1 raw f32 for the ff2 residual)
                        ln_stats(xq, qt)
                        nc.vector.tensor_scalar(
                            out=xbf1[:, qt, :], in0=xq,
                            scalar1=mv2[:, qt, 0:1],
                            scalar2=rstd[:, qt:qt + 1],
                            op0=OP.subtract, op1=OP.mult)
                        # transpose normalized qt block for ff1
                        for fc in range(KC):
                            pst = ps_tr.tile([128, 128], BF16, name="pst",
                                             tag="pst")
                            nc.tensor.transpose(
                                pst, xbf1[:, qt, fc * 128:fc * 128 + 128],
                                ident_b)
                            nc.vector.tensor_copy(out=xlnT[:, fc, qc],
                                                  in_=pst)

            xbf1_stack.close()
            wo_stack.close()
            xot_stack.close()
            av_stack.close()   # free Vaug, Vsp, ctxT
            sp_stack.close()   # free psp, kspT, qspT

            # f2T resident for qt-outer ff2 (DMA hides under ff1)
            f2_stack = contextlib.ExitStack()
            f2_pool = f2_stack.enter_context(
                tc.tile_pool(name="f2_pool", bufs=1))
            f2_s = f2_pool.tile([128, FC, D], BF16, name="f2_s")
            nc.sync.dma_start(out=f2_s, in_=f2T_r)

            # xg = xhat*g1 + (be1+b2), computed on DVE during ff1
            xg = fse.tile([128, NQT, D], F32, name="xg")

            # ============ ff1 + relu ============
            h1_stack = contextlib.ExitStack()
            h1_pool = h1_stack.enter_context(
                tc.tile_pool(name="h1_pool", bufs=1))
            h1T = h1_pool.tile([128, FC, SQ], BF16, name="h1T")
            with contextlib.ExitStack() as ph9:
                w3str = ph9.enter_context(tc.tile_pool(name="w3str", bufs=2))
                ps_f1 = ph9.enter_context(
                    tc.tile_pool(name="ps_f1", bufs=4, space="PSUM"))
                with nc.named_scope("p9_ff1"):
                    for jj in range(16):
                        wt = w3str.tile([128, KC, 256], BF16, name="w1t",
                                        tag="w3")
                        f0 = jj * 256
                        eng = nc.scalar if jj % 2 == 0 else nc.sync
                        eng.dma_start(out=wt, in_=f1T_r[:, :, f0:f0 + 256])
                        for fi in range(2):
                            dft = jj * 2 + fi
                            ps = ps_f1.tile([128, 512], F32, name="ps_f",
                                            tag="ps_f")
                            for kc in range(KC):
                                nc.tensor.matmul(
                                    ps, wt[:, kc, fi * 128:fi * 128 + 128],
                                    xlnT[:, kc, :],
                                    start=(kc == 0), stop=(kc == KC - 1))
                            nc.scalar.activation(
                                out=h1T[:, dft, :], in_=ps, func=AF.Relu,
                                bias=b1_c[:, dft:dft + 1], scale=1.0)
                        if jj < 2 * NQT and jj % 2 == 1:
                            # xg for qt = jj//2, hidden under ff1
                            qt = jj // 2
                            nc.vector.tensor_scalar(
                                out=xg[:, qt, :], in0=x1[:, qt, :],
                                scalar1=mv2[:, qt, 0:1],
                                scalar2=rstd[:, qt:qt + 1],
                                op0=OP.subtract, op1=OP.mult)
                            nc.vector.tensor_mul(xg[:, qt, :], xg[:, qt, :],
                                                 g1_bc)
                            nc.vector.tensor_add(xg[:, qt, :], xg[:, qt, :],
                                                 b12_bc)
            xln_stack.close()

            # ============ ff2 (qt-outer) + residual + LN2 + out ============
            with contextlib.ExitStack() as ph10:
                ps_f2 = ph10.enter_context(
                    tc.tile_pool(name="ps_f2", bufs=4, space="PSUM"))
                ot_pool = ph10.enter_context(
                    tc.tile_pool(name="ot_pool", bufs=2))
                with nc.named_scope("p10_ff2"):
                    for qt in range(NQT):
                        qc = slice(qt * 128, qt * 128 + 128)
                        pg0 = ps_f2.tile([128, 512], F32, name="pg", tag="pg")
                        pg1 = ps_f2.tile([128, 512], F32, name="pg", tag="pg")
                        for kc in range(FC):
                            st, sp = (kc == 0), (kc == FC - 1)
                            nc.tensor.matmul(pg0, h1T[:, kc, qc],
                                             f2_s[:, kc, 0:512],
                                             start=st, stop=sp)
                            nc.tensor.matmul(pg1, h1T[:, kc, qc],
                                             f2_s[:, kc, 512:1024],
                                             start=st, stop=sp)
                        x2 = x1[:, qt, :]
                        nc.vector.tensor_add(x2[:, 0:512], pg0,
                                             xg[:, qt, 0:512])
                        nc.vector.tensor_add(x2[:, 512:1024], pg1,
                                             xg[:, qt, 512:1024])
                        ln_stats(x2, qt)
                        ot = ot_pool.tile([128, D], F32, name="out_t",
                                          tag="out_t")
                        nc.vector.tensor_scalar(
                            out=ot, in0=x2, scalar1=mv2[:, qt, 0:1],
                            scalar2=rstd[:, qt:qt + 1],
                            op0=OP.subtract, op1=OP.mult)
                        nc.vector.tensor_mul(ot, ot, g2_bc)
                        nc.vector.tensor_add(ot, ot, be2_bc)
                        nc.scalar.dma_start(
                            out=out.ap()[qt * 128:qt * 128 + 128, :], in_=ot)
            h1_stack.close()
            f2_stack.close()

    nc.compile()
    return nc


def _prep_inputs(src, in_proj_w, in_proj_b, out_proj_w, out_proj_b,
                 Qp_w, Qp_b, Kp_w, Kp_b, Vp_w, Vp_b, lam,
                 ff1_w, ff1_b, ff2_w, ff2_b, ln1_g, ln1_b, ln2_g, ln2_b):
    import ml_dtypes
    f = np.float32
    A = lambda x: np.ascontiguousarray(x, dtype=f)
    AB = lambda x: np.ascontiguousarray(np.asarray(x, dtype=f),
                                        dtype=ml_dtypes.bfloat16)
    in_proj_w = np.asarray(in_proj_w, dtype=f)
    ff1_w = np.asarray(ff1_w, dtype=f)
    ln1_g = np.asarray(ln1_g, dtype=f)
    ln1_b = np.asarray(ln1_b, dtype=f)
    # fold ln1 gamma into ff1 weights, ln1 beta into ff1 bias
    f1_eff = ff1_w * ln1_g[None, :]            # [DFF, D]
    b1_eff = np.asarray(ff1_b, dtype=f) + ff1_w @ ln1_b

    # packed per-partition bias columns [128, 50]
    bcols = np.zeros((128, 50), dtype=f)
    qk_bias = np.asarray(in_proj_b, dtype=f)
    bcols[:, 0:8] = qk_bias[0:D].reshape(8, 128).T
    bcols[:, 8:16] = qk_bias[D:2 * D].reshape(8, 128).T
    bcols[:, 16:48] = b1_eff.reshape(32, 128).T
    bcols[0:64, 48] = np.asarray(Qp_b, dtype=f)
    bcols[0:64, 49] = np.asarray(Kp_b, dtype=f)

    # packed broadcast rows [1, 7D]: bv, bvp, bo, b12, g1, g2, be2
    brow = np.concatenate([
        qk_bias[2 * D:3 * D],
        np.asarray(Vp_b, dtype=f),
        np.asarray(out_proj_b, dtype=f),
        ln1_b + np.asarray(ff2_b, dtype=f),
        ln1_g,
        np.asarray(ln2_g, dtype=f),
        np.asarray(ln2_b, dtype=f),
    ]).reshape(1, 7 * D)

    # qkp packed partition-major: [128, kc, 2R] contiguous
    qkpT = np.concatenate([np.asarray(Qp_w).T, np.asarray(Kp_w).T], axis=1)
    qkp_pm = np.ascontiguousarray(
        qkpT.reshape(KC, 128, 2 * R).transpose(1, 0, 2).reshape(128, -1),
        dtype=f)

    shared = {
        "wqkvT": AB(in_proj_w.T),
        "woT": AB(np.asarray(out_proj_w).T),
        "vpT": AB(np.asarray(Vp_w).T),
        "qkp_pm": qkp_pm,
        "f1T": AB(f1_eff.T),
        "f2T": AB(np.asarray(ff2_w).T),
        "bcols": bcols,
        "brow": A(brow),
        "lam": A(np.asarray(lam)).reshape(1, 1),
    }
    in_maps = []
    for core in range(8):
        b, h = core // 2, core % 2
        srcb = np.asarray(src[b])
        xTb = srcb.T
        if h == 1:
            # own-query columns first (key order is irrelevant to attention)
            xTb = np.concatenate([xTb[:, SQ:], xTb[:, :SQ]], axis=1)
        m = dict(shared)
        m["xT"] = A(xTb)
        m["x_own"] = A(srcb[h * SQ:(h + 1) * SQ, :])
        in_maps.append(m)
    return in_maps


def _run(inputs, trace=False):
    if "nc" not in _cached:
        _cached["nc"] = _build()
    nc = _cached["nc"]
    in_maps = _prep_inputs(**inputs)
    res = run_bass_kernel_spmd(nc, in_maps, core_ids=list(range(8)),
                               trace=trace)
    out = np.empty((B, S, D), np.float32)
    for core in range(8):
        b, h = core // 2, core % 2
        out[b, h * SQ:(h + 1) * SQ, :] = res.results[core]["out"]
    return out, res


def kernel(**inputs) -> np.ndarray:
    out, _ = _run(inputs, trace=False)
    return out


# revision 33
# speedup vs baseline: 1.2445x; 1.1129x over previous
"""Trainium2 Bass kernel for the EnhancedEncoderLayer (dense MHA + low-rank
top-k sparse attention + FFN, two layernorms).

Sharding: 8 cores = (batch b in 0..3) x (query-half h in {0,1}). Each core
computes output rows [b, h*512:(h+1)*512, :]. K/V-side projections are
computed redundantly per batch pair (no cross-core communication).

The host permutes src[b].T columns so each core's own query tokens are
columns 0..511 (attention contracts over all keys, so key order is
irrelevant); this keeps the SPMD program identical across cores.

v2 design notes:
- v/vsp projections are x-stationary (lhsT = xT chunk), producing
  token-major Vaug/Vsp directly -- no PE transposes, no ACT copies.
- dense attention is software-pipelined: scores(h) / proj filler /
  ctx(h-1), so ACT exp latency never stalls the in-order PE queue.
- the sparse path runs in bf16: exp writes bf16 psp, the top-k threshold
  bisection scans at 2x DVE rate with 18 iterations, spmm is bf16.
- outproj+spmm+fuse+LN1 run qt-outer so LN1 overlaps matmuls; ff2 is
  qt-outer against an SBUF-resident f2T with LN2+output pipelined per qt.
- ln1 gamma/beta are folded into ff1 weights/bias host-side.
- all host tensors are laid out so every DMA is contiguous per partition.
"""
import sys
import os
import contextlib

for _p in ('/opt/trn_rl_repo',):
    if _p not in sys.path:
        sys.path.insert(0, _p)

import numpy as np
import concourse.bacc as bacc
import concourse.tile as tile
from concourse import mybir
from concourse.bass_utils import run_bass_kernel_spmd
from concourse.masks import make_identity

F32 = mybir.dt.float32
F32R = mybir.dt.float32r
BF16 = mybir.dt.bfloat16
AF = mybir.ActivationFunctionType
OP = mybir.AluOpType

B, S, D, H, R, DFF = 4, 1024, 1024, 16, 64, 4096
DH = D // H          # 64
SQ = S // 2          # 512 own queries per core
KK = max(1, int(S * 0.2))   # 204
KC = D // 128        # 8 contraction chunks over D
FC = DFF // 128      # 32 chunks over DFF
NQT = SQ // 128      # 4 query tiles
NTOK = S // 128      # 8 token tiles
BISECT_ITERS = 19
INV_SQRT = 0.125     # 1/sqrt(DH) == 1/sqrt(R)

_cached = {}


def _build():
    nc = bacc.Bacc()

    def din(name, shape, dt=F32):
        return nc.declare_dram_parameter(name, list(shape), dt, isOutput=False)

    xT = din("xT", [D, S], BF16)      # src[b].T, own-query columns first
    x_own = din("x_own", [SQ, D])     # own rows, token-major
    wqkvT = din("wqkvT", [D, 3 * D], BF16)
    woT = din("woT", [D, D], BF16)
    vpT = din("vpT", [D, D], BF16)
    qkp_pm = din("qkp_pm", [128, KC * 2 * R], BF16)   # partition-major packed
    f1T = din("f1T", [D, DFF], BF16)  # pre-scaled by ln1 gamma
    f2T = din("f2T", [DFF, D], BF16)
    # packed per-partition bias columns: [0:8]=q, [8:16]=k, [16:48]=b1_eff,
    # [48]=bqp (rows 0:64), [49]=bkp (rows 0:64)
    bcols = din("bcols", [128, 50])
    # packed broadcast rows: [bv, bvp, bo, b12, g1, g2, be2]
    brow = din("brow", [1, 7 * D])
    lam = din("lam", [1, 1])
    out = nc.declare_dram_parameter("out", [SQ, D], F32, isOutput=True)

    xT_r = xT.ap().rearrange("(kc p) s -> p kc s", p=128)
    wqkvT_r = wqkvT.ap().rearrange("(kc p) f -> p kc f", p=128)
    woT_r = woT.ap().rearrange("(kc p) f -> p kc f", p=128)
    vpT_r = vpT.ap().rearrange("(kc p) f -> p kc f", p=128)
    qkp_r = qkp_pm.ap().rearrange("p (kc f) -> p kc f", f=2 * R)
    f1T_r = f1T.ap().rearrange("(kc p) f -> p kc f", p=128)
    f2T_r = f2T.ap().rearrange("(kc p) f -> p kc f", p=128)

    with tile.TileContext(nc) as tc:
        est = contextlib.ExitStack()
        with est:
            # ---------------- constants ----------------
            consts = est.enter_context(tc.tile_pool(name="consts", bufs=1))

            ident_f = consts.tile([128, 128], F32, name="ident_f")
            make_identity(nc, ident_f)
            ident_b = consts.tile([128, 128], BF16, name="ident_b")
            nc.vector.tensor_copy(out=ident_b, in_=ident_f)

            eps_t = consts.tile([128, 1], F32, name="eps_t")
            nc.vector.memset(eps_t, 1e-5)
            ones1 = consts.tile([128, 1], F32, name="ones1")
            nc.vector.memset(ones1, 1.0)
            ones1b = consts.tile([128, 1], BF16, name="ones1b")
            nc.vector.memset(ones1b, 1.0)
            ones16b = consts.tile([128, 16], BF16, name="ones16b")
            nc.vector.memset(ones16b, 1.0)

            lam_t = consts.tile([1, 1], F32, name="lam_t")
            nc.sync.dma_start(out=lam_t, in_=lam.ap())
            sg_t = consts.tile([1, 1], F32, name="sg_t")
            nc.scalar.activation(out=sg_t, in_=lam_t, func=AF.Sigmoid)
            sig_bc = consts.tile([128, 1], F32, name="sig_bc")
            nc.gpsimd.partition_broadcast(sig_bc, sg_t)
            oms_bc = consts.tile([128, 1], F32, name="oms_bc")
            nc.vector.tensor_sub(oms_bc, ones1, sig_bc)

            # packed bias columns (one contiguous DMA)
            bcols_t = consts.tile([128, 50], F32, name="bcols_t")
            nc.gpsimd.dma_start(out=bcols_t, in_=bcols.ap())
            bq_c = bcols_t[:, 0:8]
            bk_c = bcols_t[:, 8:16]
            b1_c = bcols_t[:, 16:48]
            bqp_c = bcols_t[0:64, 48:49]
            bkp_c = bcols_t[0:64, 49:50]

            # bisect state
            bis = est.enter_context(tc.tile_pool(name="bis", bufs=1))
            lo = bis.tile([128, NQT], F32, name="lo")
            hi = bis.tile([128, NQT], F32, name="hi")
            mid = bis.tile([128, NQT], F32, name="mid")
            cnts = bis.tile([128, NQT], F32, name="cnts")
            pred = bis.tile([128, NQT], mybir.dt.uint32, name="pred")
            rs_sp = bis.tile([128, NQT], F32, name="rs_sp")
            rcp_sp = bis.tile([128, NQT], F32, name="rcp_sp")

            # long-lived activations
            sp_stack = contextlib.ExitStack()
            sp_pool = sp_stack.enter_context(
                tc.tile_pool(name="sp_pool", bufs=1))
            psp = sp_pool.tile([128, NQT, S], BF16, name="psp")
            kspT = sp_pool.tile([64, S], F32R, name="kspT")
            qspT = sp_pool.tile([64, SQ], F32R, name="qspT")

            av_stack = contextlib.ExitStack()
            av_pool = av_stack.enter_context(
                tc.tile_pool(name="av_pool", bufs=1))
            Vaug = av_pool.tile([128, NTOK, H * (DH + 1)], BF16, name="Vaug")
            Vsp = av_pool.tile([128, NTOK, D], BF16, name="Vsp")
            ctxT = av_pool.tile([128, KC, SQ], BF16, name="ctxT")

            Vaug_h = Vaug.rearrange("p t (h c) -> p t h c", c=DH + 1)
            for t in range(NTOK):
                nc.vector.tensor_copy(out=Vaug_h[:, t, :, DH:DH + 1],
                                      in_=ones16b)

            xot_stack = contextlib.ExitStack()
            xot_pool = xot_stack.enter_context(tc.tile_pool(name="xot_pool",
                                                            bufs=1))
            xot = xot_pool.tile([128, NQT, D], F32, name="xot")

            # out_proj weights (DMA issued later, after the startup crunch)
            wo_stack = contextlib.ExitStack()
            wo_pool = wo_stack.enter_context(
                tc.tile_pool(name="wo_pool", bufs=1))
            woT_s = wo_pool.tile([128, KC, D], BF16, name="woT_s")

            # =========== phase 0/1: input loads, sparse + v projections ====
            xbf_stack = contextlib.ExitStack()
            xbf_pool = xbf_stack.enter_context(
                tc.tile_pool(name="xbf_pool", bufs=1))
            xbf = xbf_pool.tile([128, KC, S], BF16, name="xbf")

            # early broadcast rows: bv, bvp, sig*bo
            early_stack = contextlib.ExitStack()
            early_bc = early_stack.enter_context(
                tc.tile_pool(name="early_bc", bufs=1))
            with contextlib.ExitStack() as brow_stack:
                brow_pool = brow_stack.enter_context(
                    tc.tile_pool(name="brow_pool", bufs=1))
                brow_t = brow_pool.tile([1, 3 * D], F32, name="brow_t")
                nc.gpsimd.dma_start(out=brow_t, in_=brow.ap()[:, 0:3 * D])
                bv_bc = early_bc.tile([128, D], F32, name="bv_bc")
                nc.gpsimd.partition_broadcast(bv_bc, brow_t[:, 0 * D:1 * D])
                bvp_bc = early_bc.tile([128, D], F32, name="bvp_bc")
                nc.gpsimd.partition_broadcast(bvp_bc, brow_t[:, 1 * D:2 * D])
                bo_sig = early_bc.tile([128, D], F32, name="bo_sig")
                nc.gpsimd.partition_broadcast(bo_sig, brow_t[:, 2 * D:3 * D])
                nc.vector.tensor_scalar_mul(bo_sig, bo_sig, sig_bc)

            with contextlib.ExitStack() as ph0:
                wsp_pool = ph0.enter_context(
                    tc.tile_pool(name="wsp_pool", bufs=1))
                ps_proj = ph0.enter_context(
                    tc.tile_pool(name="ps_proj", bufs=3, space="PSUM"))

                qkpt = wsp_pool.tile([128, KC, 2 * R], BF16, name="qkpt")
                nc.sync.dma_start(out=qkpt, in_=qkp_r)
                qpt = qkpt[:, :, 0:R]
                kpt = qkpt[:, :, R:2 * R]
                for kc in range(KC):
                    eng = nc.scalar if kc % 2 == 0 else nc.sync
                    eng.dma_start(out=xbf[:, kc, :], in_=xT_r[:, kc, :])

                # ---- sparse projections + scores ----
                with nc.named_scope("p0_ksp_qsp"):
                    for nh in range(2):
                        ps = ps_proj.tile([128, 512], F32, name="ps",
                                          tag="ps")
                        for kc in range(KC):
                            nc.tensor.matmul(
                                ps[0:64, :], kpt[:, kc, :],
                                xbf[:, kc, nh * 512:nh * 512 + 512],
                                start=(kc == 0), stop=(kc == KC - 1))
                        nc.scalar.activation(
                            out=kspT[:, nh * 512:nh * 512 + 512],
                            in_=ps[0:64, :], func=AF.Identity, bias=bkp_c,
                            scale=1.0)
                    ps = ps_proj.tile([128, 512], F32, name="ps", tag="ps")
                    for kc in range(KC):
                        nc.tensor.matmul(ps[0:64, :], qpt[:, kc, :],
                                         xbf[:, kc, 0:SQ],
                                         start=(kc == 0), stop=(kc == KC - 1))
                    nc.scalar.activation(out=qspT, in_=ps[0:64, :],
                                         func=AF.Identity, bias=bqp_c,
                                         scale=1.0)

                with nc.named_scope("p2_ssp"):
                    for qt in range(NQT):
                        for nh in range(2):
                            ps = ps_proj.tile([128, 512], F32, name="ps",
                                              tag="ps")
                            nc.tensor.matmul(
                                ps, qspT[:, qt * 128:qt * 128 + 128],
                                kspT[:, nh * 512:nh * 512 + 512],
                                start=True, stop=True)
                            nc.scalar.activation(
                                out=psp[:, qt, nh * 512:nh * 512 + 512],
                                in_=ps, func=AF.Exp, scale=INV_SQRT)

                # own-token residual (+ sig*bo)
                for qt in range(NQT):
                    nc.scalar.dma_start(
                        out=xot[:, qt, :],
                        in_=x_own.ap()[qt * 128:qt * 128 + 128, :])
                    nc.gpsimd.tensor_add(xot[:, qt, :], xot[:, qt, :],
                                         bo_sig)

            # ---- v/vsp x-stationary projections -> token-major ----
            with contextlib.ExitStack() as ph4:
                # reopened weight pool (wv_s, wvp_s still live via av? no --
                # keep them in this scope)
                wv_pool2 = ph4.enter_context(
                    tc.tile_pool(name="wv_pool2", bufs=1))
                wv_s = wv_pool2.tile([128, KC, D], BF16, name="wv_s2")
                wvp_s = wv_pool2.tile([128, KC, D], BF16, name="wvp_s2")
                for kc in range(KC):
                    nc.sync.dma_start(out=wv_s[:, kc, :],
                                      in_=wqkvT_r[:, kc, 2 * D:3 * D])
                    nc.sync.dma_start(out=wvp_s[:, kc, :],
                                      in_=vpT_r[:, kc, :])
                ps_v = ph4.enter_context(
                    tc.tile_pool(name="ps_v", bufs=8, space="PSUM"))
                with nc.named_scope("p4_v"):
                    for t in range(NTOK):
                        pva0 = ps_v.tile([128, 512], F32, name="pv", tag="pv")
                        pva1 = ps_v.tile([128, 512], F32, name="pv", tag="pv")
                        pvs0 = ps_v.tile([128, 512], F32, name="pv", tag="pv")
                        pvs1 = ps_v.tile([128, 512], F32, name="pv", tag="pv")
                        for kc in range(KC):
                            xck = xbf[:, kc, t * 128:t * 128 + 128]
                            st = (kc == 0)
                            sp = (kc == KC - 1)
                            nc.tensor.matmul(pva0, xck, wv_s[:, kc, 0:512],
                                             start=st, stop=sp)
                            nc.tensor.matmul(pva1, xck, wv_s[:, kc, 512:1024],
                                             start=st, stop=sp)
                            nc.tensor.matmul(pvs0, xck, wvp_s[:, kc, 0:512],
                                             start=st, stop=sp)
                            nc.tensor.matmul(pvs1, xck, wvp_s[:, kc, 512:1024],
                                             start=st, stop=sp)
                        nc.vector.tensor_add(
                            Vaug_h[:, t, 0:8, 0:DH], pva0, bv_bc[:, 0:512])
                        nc.vector.tensor_add(
                            Vaug_h[:, t, 8:16, 0:DH], pva1, bv_bc[:, 512:1024])
                        nc.vector.tensor_add(
                            Vsp[:, t, 0:512], pvs0, bvp_bc[:, 0:512])
                        nc.vector.tensor_add(
                            Vsp[:, t, 512:1024], pvs1, bvp_bc[:, 512:1024])
            early_stack.close()   # free bv_bc, bvp_bc, bo_sig

            # out_proj weights resident (used in p6)
            nc.sync.dma_start(out=woT_s, in_=woT_r)

            # bisect scratch: lives until after the masking pass
            scr_stack = contextlib.ExitStack()
            scr_pool = scr_stack.enter_context(
                tc.tile_pool(name="scr", bufs=4))

            def bisect_iter():
                # one threshold-bisection step; qt 0-2 scan on DVE, qt 3 on
                # GpSimd (SBUF-only engine, otherwise idle here)
                nc.vector.tensor_add(mid, lo, hi)
                nc.vector.tensor_scalar_mul(mid, mid, 0.5)
                for qt in range(NQT):
                    scr = scr_pool.tile([128, S], BF16, name="scr",
                                        tag="scr")
                    nc.vector.scalar_tensor_tensor(
                        out=scr, in0=psp[:, qt, :],
                        scalar=mid[:, qt:qt + 1],
                        in1=ones1b.to_broadcast([128, S]),
                        op0=OP.is_ge, op1=OP.mult,
                        accum_out=cnts[:, qt:qt + 1])
                nc.vector.tensor_scalar(out=pred, in0=cnts,
                                        scalar1=float(KK),
                                        scalar2=None, op0=OP.is_ge)
                nc.vector.copy_predicated(lo, pred, mid)
                nc.vector.tensor_scalar(out=pred, in0=cnts,
                                        scalar1=float(KK),
                                        scalar2=None, op0=OP.is_lt)
                nc.vector.copy_predicated(hi, pred, mid)

            # ======== phase 5: k/q projections + pipelined attention =======
            kq_stack = contextlib.ExitStack()
            kq_pool = kq_stack.enter_context(
                tc.tile_pool(name="kq_pool", bufs=1))
            kT = kq_pool.tile([128, KC, S], BF16, name="kT")
            qT = kq_pool.tile([128, KC, SQ], BF16, name="qT")
            with contextlib.ExitStack() as ph5:
                wstr = ph5.enter_context(tc.tile_pool(name="wstr", bufs=3))
                pt_pool = ph5.enter_context(
                    tc.tile_pool(name="pt_pool", bufs=16))
                rc_pool = ph5.enter_context(
                    tc.tile_pool(name="rc_pool", bufs=2))
                ps_kq = ph5.enter_context(
                    tc.tile_pool(name="ps_kq", bufs=3, space="PSUM"))
                ps_attn = ph5.enter_context(
                    tc.tile_pool(name="ps_attn", bufs=3, space="PSUM"))
                ps_ctx = ph5.enter_context(
                    tc.tile_pool(name="ps_ctx", bufs=2, space="PSUM"))

                pts = {}

                def proj_piece(ft):
                    wk = wstr.tile([128, KC, 128], BF16, name="wk", tag="wk")
                    nc.sync.dma_start(
                        out=wk, in_=wqkvT_r[:, :, D + ft * 128:D + ft * 128 + 128])
                    wq = wstr.tile([128, KC, 128], BF16, name="wq", tag="wq")
                    nc.sync.dma_start(
                        out=wq, in_=wqkvT_r[:, :, ft * 128:ft * 128 + 128])
                    for nh in range(2):
                        ps = ps_kq.tile([128, 512], F32, name="pkq",
                                        tag="pkq")
                        for kc in range(KC):
                            nc.tensor.matmul(
                                ps, wk[:, kc, :],
                                xbf[:, kc, nh * 512:nh * 512 + 512],
                                start=(kc == 0), stop=(kc == KC - 1))
                        nc.scalar.activation(
                            out=kT[:, ft, nh * 512:nh * 512 + 512],
                            in_=ps, func=AF.Identity,
                            bias=bk_c[:, ft:ft + 1], scale=1.0)
                    ps = ps_kq.tile([128, 512], F32, name="pkq", tag="pkq")
                    for kc in range(KC):
                        nc.tensor.matmul(ps, wq[:, kc, :], xbf[:, kc, 0:SQ],
                                         start=(kc == 0), stop=(kc == KC - 1))
                    nc.scalar.activation(
                        out=qT[:, ft, :], in_=ps, func=AF.Identity,
                        bias=bq_c[:, ft:ft + 1], scale=1.0)

                def scores(hh):
                    po = 64 * (hh % 2)
                    ft = hh // 2
                    tiles = []
                    for t in range(NTOK):
                        ps = ps_attn.tile([128, 512], F32, name="ps_s",
                                          tag="ps_s")
                        nc.tensor.matmul(
                            ps, kT[po:po + 64, ft, t * 128:t * 128 + 128],
                            qT[po:po + 64, ft, :], start=True, stop=True)
                        pt = pt_pool.tile([128, 512], BF16, name="pT",
                                          tag="pT")
                        nc.scalar.activation(out=pt, in_=ps, func=AF.Exp,
                                             scale=INV_SQRT)
                        tiles.append(pt)
                    pts[hh] = tiles

                def ctx(hh):
                    po = 64 * (hh % 2)
                    ft = hh // 2
                    pctx = ps_ctx.tile([128, 512], F32, name="ps_c",
                                       tag="ps_c")
                    for t in range(NTOK):
                        nc.tensor.matmul(
                            pctx[0:65, :], Vaug_h[:, t, hh, 0:DH + 1],
                            pts[hh][t], start=(t == 0), stop=(t == NTOK - 1))
                    rsr = rc_pool.tile([1, 512], F32, name="rsr", tag="rsr")
                    nc.vector.tensor_copy(out=rsr, in_=pctx[64:65, :])
                    rch = rc_pool.tile([1, 512], F32, name="rch", tag="rch")
                    nc.vector.reciprocal_approx_fast(out=rch, in_=rsr)
                    rb = rc_pool.tile([64, 512], F32, name="rb", tag="rb")
                    nc.gpsimd.partition_broadcast(rb, rch)
                    nc.vector.tensor_mul(out=ctxT[po:po + 64, ft, :],
                                         in0=pctx[0:64, :], in1=rb)
                    del pts[hh]

                with nc.named_scope("p5_kq_attn"):
                    nc.vector.memset(lo, 0.0)
                    nc.vector.memset(hi, 16.0)
                    proj_piece(0)
                    proj_piece(1)
                    bisect_iter()
                    for hh in range(H):
                        if hh % 2 == 0 and hh // 2 + 2 < KC:
                            proj_piece(hh // 2 + 2)
                        scores(hh)
                        if hh >= 1:
                            ctx(hh - 1)
                        bisect_iter()
                    ctx(H - 1)

                # final masking + renorm scale for the sparse path
                with nc.named_scope("p3_mask"):
                    for qt in range(NQT):
                        nc.vector.scalar_tensor_tensor(
                            out=psp[:, qt, :], in0=psp[:, qt, :],
                            scalar=lo[:, qt:qt + 1],
                            in1=psp[:, qt, :], op0=OP.is_ge, op1=OP.mult,
                            accum_out=rs_sp[:, qt:qt + 1])
                    nc.vector.tensor_scalar(out=rs_sp, in0=rs_sp,
                                            scalar1=1e-9, scalar2=None,
                                            op0=OP.add)
                    nc.vector.reciprocal(rcp_sp, rs_sp)
                    nc.vector.tensor_scalar_mul(rcp_sp, rcp_sp, oms_bc)

            kq_stack.close()    # free kT, qT
            scr_stack.close()
            xbf_stack.close()   # free xbf

            # ========= phase 6: outproj + spmm + fuse + LN1 (qt-outer) =====
            # late broadcast rows: b12, g1, g2, be2 (right-side stack)
            late_bc = est.enter_context(
                tc.tile_pool(name="late_bc", bufs=1, side="right"))
            with contextlib.ExitStack() as brow_stack:
                brow_pool = brow_stack.enter_context(
                    tc.tile_pool(name="brow_pool2", bufs=1, side="right"))
                brow_t = brow_pool.tile([1, 4 * D], F32, name="brow_t2")
                nc.gpsimd.dma_start(out=brow_t,
                                    in_=brow.ap()[:, 3 * D:7 * D])
                b12_bc = late_bc.tile([128, D], F32, name="b12_bc")
                nc.gpsimd.partition_broadcast(b12_bc, brow_t[:, 0:D])
                g1_bc = late_bc.tile([128, D], F32, name="g1_bc")
                nc.gpsimd.partition_broadcast(g1_bc, brow_t[:, D:2 * D])
                g2_bc = late_bc.tile([128, D], F32, name="g2_bc")
                nc.gpsimd.partition_broadcast(g2_bc, brow_t[:, 2 * D:3 * D])
                be2_bc = late_bc.tile([128, D], F32, name="be2_bc")
                nc.gpsimd.partition_broadcast(be2_bc, brow_t[:, 3 * D:4 * D])

            fse = est.enter_context(tc.tile_pool(name="fse", bufs=1,
                                                 side="right"))
            x1 = fse.tile([128, NQT, D], F32, name="x1")
            mv2 = fse.tile([128, NQT, 2], F32, name="mv2")
            stats = fse.tile([128, NQT, 2, 6], F32, name="stats")
            sd = fse.tile([128, NQT], F32, name="sd")
            rstd = fse.tile([128, NQT], F32, name="rstd")

            xln_stack = contextlib.ExitStack()
            xlnT_pool = xln_stack.enter_context(
                tc.tile_pool(name="xlnT_pool", bufs=1, side="right"))
            xlnT = xlnT_pool.tile([128, KC, SQ], BF16, name="xlnT")
            xbf1_stack = contextlib.ExitStack()
            xbf1_pool = xbf1_stack.enter_context(
                tc.tile_pool(name="xbf1_pool", bufs=1, side="right"))
            xbf1 = xbf1_pool.tile([128, NQT, D], BF16, name="xbf1")

            def ln_stats(src_ap, qt):
                for half in range(2):
                    nc.vector.bn_stats(
                        out=stats[:, qt, half, :],
                        in_=src_ap[:, half * 512:half * 512 + 512])
                nc.vector.bn_aggr(out=mv2[:, qt, :], in_=stats[:, qt])
                nc.scalar.activation(out=sd[:, qt:qt + 1],
                                     in_=mv2[:, qt, 1:2], func=AF.Sqrt,
                                     bias=eps_t, scale=1.0)
                nc.vector.reciprocal(rstd[:, qt:qt + 1], sd[:, qt:qt + 1])

            with contextlib.ExitStack() as ph6:
                pm_pool = ph6.enter_context(tc.tile_pool(name="pm_pool",
                                                         bufs=2))
                ps_o = ph6.enter_context(
                    tc.tile_pool(name="ps_o", bufs=4, space="PSUM"))
                ps_sp = ph6.enter_context(
                    tc.tile_pool(name="ps_sp", bufs=2, space="PSUM"))
                ps_tr = ph6.enter_context(
                    tc.tile_pool(name="ps_tr", bufs=2, space="PSUM"))
                def xln_transpose(qt):
                    # transpose normalized qt block for ff1 (lagged one qt
                    # so the PE never waits on LN1's DVE chain)
                    qc = slice(qt * 128, qt * 128 + 128)
                    for fc in range(KC):
                        pst = ps_tr.tile([128, 128], BF16, name="pst",
                                         tag="pst")
                        nc.tensor.transpose(
                            pst, xbf1[:, qt, fc * 128:fc * 128 + 128],
                            ident_b)
                        nc.vector.tensor_copy(out=xlnT[:, fc, qc],
                                              in_=pst)

                with nc.named_scope("p6_fuse"):
                    for qt in range(NQT):
                        qc = slice(qt * 128, qt * 128 + 128)
                        # out_proj (2 halves, ctxT-stationary)
                        po0 = ps_o.tile([128, 512], F32, name="po", tag="po")
                        po1 = ps_o.tile([128, 512], F32, name="po", tag="po")
                        for kc in range(KC):
                            st, sp = (kc == 0), (kc == KC - 1)
                            nc.tensor.matmul(po0, ctxT[:, kc, qc],
                                             woT_s[:, kc, 0:512],
                                             start=st, stop=sp)
                            nc.tensor.matmul(po1, ctxT[:, kc, qc],
                                             woT_s[:, kc, 512:1024],
                                             start=st, stop=sp)
                        # masked-p transposes for this qt
                        pmt = pm_pool.tile([128, NTOK, 128], BF16, name="pmt",
                                           tag="pmt")
                        for t in range(NTOK):
                            pst = ps_tr.tile([128, 128], BF16, name="pst",
                                             tag="pst")
                            nc.tensor.transpose(
                                pst, psp[:, qt, t * 128:t * 128 + 128],
                                ident_b)
                            nc.vector.tensor_copy(out=pmt[:, t, :], in_=pst)
                        # spmm (2 halves)
                        sp0 = ps_sp.tile([128, 512], F32, name="psp2",
                                         tag="psp2")
                        sp1 = ps_sp.tile([128, 512], F32, name="psp2",
                                         tag="psp2")
                        for t in range(NTOK):
                            st, spl = (t == 0), (t == NTOK - 1)
                            nc.tensor.matmul(sp0, pmt[:, t, :],
                                             Vsp[:, t, 0:512],
                                             start=st, stop=spl)
                            nc.tensor.matmul(sp1, pmt[:, t, :],
                                             Vsp[:, t, 512:1024],
                                             start=st, stop=spl)
                        if qt >= 1:
                            xln_transpose(qt - 1)
                        # fuse on DVE: x1 = sig*dense + rcp*spmm + xot
                        xq = x1[:, qt, :]
                        nc.vector.tensor_scalar(
                            out=xq[:, 0:512], in0=po0, scalar1=sig_bc,
                            scalar2=None, op0=OP.mult)
                        nc.vector.tensor_scalar(
                            out=xq[:, 512:1024], in0=po1, scalar1=sig_bc,
                            scalar2=None, op0=OP.mult)
                        nc.vector.tensor_add(xq, xq, xot[:, qt, :])
                        nc.vector.scalar_tensor_tensor(
                            out=xq[:, 0:512], in0=sp0,
                            scalar=rcp_sp[:, qt:qt + 1],
                            in1=xq[:, 0:512], op0=OP.mult, op1=OP.add)
                        nc.vector.scalar_tensor_tensor(
                            out=xq[:, 512:1024], in0=sp1,
                            scalar=rcp_sp[:, qt:qt + 1],
                            in1=xq[:, 512:1024], op0=OP.mult, op1=OP.add)
                        # LN1 (keep x1 raw f32 for the ff2 residual)
                        ln_stats(xq, qt)
                        nc.vector.tensor_scalar(
                            out=xbf1[:, qt, :], in0=xq,
                            scalar1=mv2[:, qt, 0:1],
                            scalar2=rstd[:, qt:qt + 1],
                            op0=OP.subtract, op1=OP.mult)
                    xln_transpose(NQT - 1)

            xbf1_stack.close()
            wo_stack.close()
            xot_stack.close()
            av_stack.close()   # free Vaug, Vsp, ctxT
            sp_stack.close()   # free psp, kspT, qspT

            # f2T resident for qt-outer ff2 (DMA hides under ff1)
            f2_stack = contextlib.ExitStack()
            f2_pool = f2_stack.enter_context(
                tc.tile_pool(name="f2_pool", bufs=1))
            f2_s = f2_pool.tile([128, FC, D], BF16, name="f2_s")
            nc.sync.dma_start(out=f2_s, in_=f2T_r)

            # xg = xhat*g1 + (be1+b2), computed on DVE during ff1
            xg = fse.tile([128, NQT, D], F32, name="xg")

            # ============ ff1 + relu ============
            h1_stack = contextlib.ExitStack()
            h1_pool = h1_stack.enter_context(
                tc.tile_pool(name="h1_pool", bufs=1))
            h1T = h1_pool.tile([128, FC, SQ], BF16, name="h1T")
            with contextlib.ExitStack() as ph9:
                w3str = ph9.enter_context(tc.tile_pool(name="w3str", bufs=2))
                ps_f1 = ph9.enter_context(
                    tc.tile_pool(name="ps_f1", bufs=4, space="PSUM"))
                with nc.named_scope("p9_ff1"):
                    for jj in range(16):
                        wt = w3str.tile([128, KC, 256], BF16, name="w1t",
                                        tag="w3")
                        f0 = jj * 256
                        eng = nc.scalar if jj % 2 == 0 else nc.sync
                        eng.dma_start(out=wt, in_=f1T_r[:, :, f0:f0 + 256])
                        for fi in range(2):
                            dft = jj * 2 + fi
                            ps = ps_f1.tile([128, 512], F32, name="ps_f",
                                            tag="ps_f")
                            for kc in range(KC):
                                nc.tensor.matmul(
                                    ps, wt[:, kc, fi * 128:fi * 128 + 128],
                                    xlnT[:, kc, :],
                                    start=(kc == 0), stop=(kc == KC - 1))
                            nc.scalar.activation(
                                out=h1T[:, dft, :], in_=ps, func=AF.Relu,
                                bias=b1_c[:, dft:dft + 1], scale=1.0)
                        if jj < 2 * NQT and jj % 2 == 1:
                            # xg for qt = jj//2, hidden under ff1
                            qt = jj // 2
                            nc.vector.tensor_scalar(
                                out=xg[:, qt, :], in0=x1[:, qt, :],
                                scalar1=mv2[:, qt, 0:1],
                                scalar2=rstd[:, qt:qt + 1],
                                op0=OP.subtract, op1=OP.mult)
                            nc.vector.tensor_mul(xg[:, qt, :], xg[:, qt, :],
                                                 g1_bc)
                            nc.vector.tensor_add(xg[:, qt, :], xg[:, qt, :],
                                                 b12_bc)
            xln_stack.close()

            # ============ ff2 (qt-outer) + residual + LN2 + out ============
            with contextlib.ExitStack() as ph10:
                ps_f2 = ph10.enter_context(
                    tc.tile_pool(name="ps_f2", bufs=4, space="PSUM"))
                ot_pool = ph10.enter_context(
                    tc.tile_pool(name="ot_pool", bufs=2))
                with nc.named_scope("p10_ff2"):
                    for qt in range(NQT):
                        qc = slice(qt * 128, qt * 128 + 128)
                        pg0 = ps_f2.tile([128, 512], F32, name="pg", tag="pg")
                        pg1 = ps_f2.tile([128, 512], F32, name="pg", tag="pg")
                        for kc in range(FC):
                            st, sp = (kc == 0), (kc == FC - 1)
                            nc.tensor.matmul(pg0, h1T[:, kc, qc],
                                             f2_s[:, kc, 0:512],
                                             start=st, stop=sp)
                            nc.tensor.matmul(pg1, h1T[:, kc, qc],
                                             f2_s[:, kc, 512:1024],
                                             start=st, stop=sp)
                        x2 = x1[:, qt, :]
                        nc.vector.tensor_add(x2[:, 0:512], pg0,
                                             xg[:, qt, 0:512])
                        nc.vector.tensor_add(x2[:, 512:1024], pg1,
                                             xg[:, qt, 512:1024])
                        ln_stats(x2, qt)
                        ot = ot_pool.tile([128, D], F32, name="out_t",
                                          tag="out_t")
                        nc.vector.tensor_scalar(
                            out=ot, in0=x2, scalar1=mv2[:, qt, 0:1],
                            scalar2=rstd[:, qt:qt + 1],
                            op0=OP.subtract, op1=OP.mult)
                        nc.vector.tensor_mul(ot, ot, g2_bc)
                        nc.vector.tensor_add(ot, ot, be2_bc)
                        nc.scalar.dma_start(
                            out=out.ap()[qt * 128:qt * 128 + 128, :], in_=ot)
            h1_stack.close()
            f2_stack.close()

    nc.compile()
    return nc


def _prep_inputs(src, in_proj_w, in_proj_b, out_proj_w, out_proj_b,
                 Qp_w, Qp_b, Kp_w, Kp_b, Vp_w, Vp_b, lam,
                 ff1_w, ff1_b, ff2_w, ff2_b, ln1_g, ln1_b, ln2_g, ln2_b):
    import ml_dtypes
    f = np.float32
    A = lambda x: np.ascontiguousarray(x, dtype=f)
    AB = lambda x: np.ascontiguousarray(np.asarray(x, dtype=f),
                                        dtype=ml_dtypes.bfloat16)
    in_proj_w = np.asarray(in_proj_w, dtype=f)
    ff1_w = np.asarray(ff1_w, dtype=f)
    ln1_g = np.asarray(ln1_g, dtype=f)
    ln1_b = np.asarray(ln1_b, dtype=f)
    # fold ln1 gamma into ff1 weights, ln1 beta into ff1 bias
    f1_eff = ff1_w * ln1_g[None, :]            # [DFF, D]
    b1_eff = np.asarray(ff1_b, dtype=f) + ff1_w @ ln1_b

    # packed per-partition bias columns [128, 50]
    bcols = np.zeros((128, 50), dtype=f)
    qk_bias = np.asarray(in_proj_b, dtype=f)
    bcols[:, 0:8] = qk_bias[0:D].reshape(8, 128).T
    bcols[:, 8:16] = qk_bias[D:2 * D].reshape(8, 128).T
    bcols[:, 16:48] = b1_eff.reshape(32, 128).T
    bcols[0:64, 48] = np.asarray(Qp_b, dtype=f)
    bcols[0:64, 49] = np.asarray(Kp_b, dtype=f)

    # packed broadcast rows [1, 7D]: bv, bvp, bo, b12, g1, g2, be2
    brow = np.concatenate([
        qk_bias[2 * D:3 * D],
        np.asarray(Vp_b, dtype=f),
        np.asarray(out_proj_b, dtype=f),
        ln1_b + np.asarray(ff2_b, dtype=f),
        ln1_g,
        np.asarray(ln2_g, dtype=f),
        np.asarray(ln2_b, dtype=f),
    ]).reshape(1, 7 * D)

    # qkp packed partition-major: [128, kc, 2R] contiguous
    qkpT = np.concatenate([np.asarray(Qp_w).T, np.asarray(Kp_w).T], axis=1)
    qkp_pm = np.ascontiguousarray(
        np.asarray(qkpT, dtype=f).reshape(KC, 128, 2 * R)
        .transpose(1, 0, 2).reshape(128, -1),
        dtype=ml_dtypes.bfloat16)

    shared = {
        "wqkvT": AB(in_proj_w.T),
        "woT": AB(np.asarray(out_proj_w).T),
        "vpT": AB(np.asarray(Vp_w).T),
        "qkp_pm": qkp_pm,
        "f1T": AB(f1_eff.T),
        "f2T": AB(np.asarray(ff2_w).T),
        "bcols": bcols,
        "brow": A(brow),
        "lam": A(np.asarray(lam)).reshape(1, 1),
    }
    in_maps = []
    for core in range(8):
        b, h = core // 2, core % 2
        srcb = np.asarray(src[b])
        xTb = srcb.T
        if h == 1:
            # own-query columns first (key order is irrelevant to attention)
            xTb = np.concatenate([xTb[:, SQ:], xTb[:, :SQ]], axis=1)
        m = dict(shared)
        m["xT"] = AB(xTb)
        m["x_own"] = A(srcb[h * SQ:(h + 1) * SQ, :])
        in_maps.append(m)
    return in_maps


def _run(inputs, trace=False):
    if "nc" not in _cached:
        _cached["nc"] = _build()
    nc = _cached["nc"]
    in_maps = _prep_inputs(**inputs)
    res = run_bass_kernel_spmd(nc, in_maps, core_ids=list(range(8)),
                               trace=trace)
    out = np.empty((B, S, D), np.float32)
    for core in range(8):
        b, h = core // 2, core % 2
        out[b, h * SQ:(h + 1) * SQ, :] = res.results[core]["out"]
    return out, res


def kernel(**inputs) -> np.ndarray:
    out, _ = _run(inputs, trace=False)
    return out
